# revision 1
# baseline (speedup 1.0000x reference)
"""BitLinear Trainium2 kernel.

Computes, for input [N, IN_F], weight [OUT_F, IN_F], bias/beta [OUT_F], gamma [IN_F]:
    scale_i = max_k |input[i, k]|                         (per-row quant scale)
    out[i, j] = sum_k sign(input[i,k]) * (scale_i / gamma[k]) * sign(weight[j,k])
    out = (out + bias) * beta

Strategy: data-parallel shard input rows across 8 NeuronCores; every core holds
the full weight, host-pre-transposed to wT [K, J] and cast to bf16 (sign() is
bit-invariant under the fp32->bf16 cast, so device results are unchanged; the
cast only halves weight DMA traffic).  The sign matmul runs on the PE array in
fp8 DoubleRow (exact: operands are +-1, integer accumulation in fp32 PSUM)
when gamma == 1, else bf16 with 1/gamma folded into the quantized weight.
Activations are sign-quantized on ACT, transposed k-major on the PE
(transpose-mode matmuls into PSUM, one big DVE copy back).  The per-row scale
is applied on the PSUM eviction path as a per-partition scalar multiply.
"""

import os
import sys
import numpy as np
from contextlib import ExitStack

sys.path.insert(0, "/opt/trn_rl_repo")

N_FULL, IN_F, OUT_F = 8192, 2048, 2048
N_CORES = 8
P = 128
NJ = 512  # matmul output column chunk (one PSUM bank)
PSJ = 1024  # psum tile width (2 banks)


def build_program(M, K, J, mode="fp8", fold_gamma=False, apply_bb=False, loop_n=0):
    """Single-core Bass program for an [M, K] x [K, J] BitLinear shard.

    DRAM inputs:  x [M, K] f32, wT [K, J] bf16 (pre-transposed, pre-cast
    weight), optionally ig [K, 1] f32 (1/gamma), optionally bb [2, J] f32
    (row 0: beta, row 1: bias*beta).  Output: out [M, J] f32.

    loop_n > 0 wraps the whole body in a device-side For loop (for timing).
    """
    import concourse.bass as bass
    import concourse.tile as tile
    from concourse import bacc, mybir
    from concourse.masks import make_identity

    assert M % P == 0 and K % P == 0 and J % PSJ == 0
    n_mt, n_kt, n_nj = M // P, K // P, J // NJ
    fp8 = mode == "fp8"
    if fp8:
        assert not fold_gamma and n_kt % 2 == 0
    cdt = mybir.dt.float8e4 if fp8 else mybir.dt.bfloat16
    f32 = mybir.dt.float32
    bf16 = mybir.dt.bfloat16

    nc = bacc.Bacc("TRN2", target_bir_lowering=False, debug=False)
    x_d = nc.dram_tensor("x", [M, K], f32, kind="ExternalInput")
    wT_d = nc.dram_tensor("wT", [K, J], bf16, kind="ExternalInput")
    ig_d = (
        nc.dram_tensor("ig", [K, 1], f32, kind="ExternalInput") if fold_gamma else None
    )
    bb_d = (
        nc.dram_tensor("bb", [2, J], f32, kind="ExternalInput") if apply_bb else None
    )
    # bf16 output: ~0.2% relative error, halves output DMA traffic; the host
    # upcasts back to fp32.
    odt = f32 if apply_bb else bf16
    out_d = nc.dram_tensor("out", [M, J], odt, kind="ExternalOutput")

    with tile.TileContext(nc) as tc, ExitStack() as ctx:
        aT_pool = ctx.enter_context(tc.tile_pool(name="aT", bufs=1))
        act_pool = ctx.enter_context(tc.tile_pool(name="act", bufs=3))
        asg_pool = ctx.enter_context(tc.tile_pool(name="asg", bufs=2))
        const_pool = ctx.enter_context(tc.tile_pool(name="const", bufs=1))
        scale_pool = ctx.enter_context(tc.tile_pool(name="scalep", bufs=1))
        wraw_pool = ctx.enter_context(tc.tile_pool(name="wraw", bufs=3))
        wq_pool = ctx.enter_context(tc.tile_pool(name="wq", bufs=1))
        out_pool = ctx.enter_context(tc.tile_pool(name="outp", bufs=3))
        psum_pool = ctx.enter_context(tc.tile_pool(name="psum", bufs=3, space="PSUM"))
        tpsum_pool = ctx.enter_context(tc.tile_pool(name="tpsum", bufs=1, space="PSUM"))

        # ---- constants (outside any timing loop) ----
        ident = const_pool.tile([P, P], bf16, name="ident")
        make_identity(nc, ident)

        beta_bc = bbeta_bc = None
        if apply_bb:
            beta_bc = const_pool.tile([P, J], f32, name="beta_bc")
            bbeta_bc = const_pool.tile([P, J], f32, name="bbeta_bc")
            nc.sync.dma_start(beta_bc[:], bb_d[0:1, :].broadcast_to([P, J]))
            nc.sync.dma_start(bbeta_bc[:], bb_d[1:2, :].broadcast_to([P, J]))

        igs = []
        if fold_gamma:
            for kt in range(n_kt):
                ig_t = const_pool.tile([P, 1], f32, name=f"ig{kt}", tag=f"ig{kt}")
                nc.sync.dma_start(ig_t[:], ig_d[kt * P : (kt + 1) * P, :])
                igs.append(ig_t)

        def body():
            scale_all = scale_pool.tile([P, n_mt], f32, name="scale_all")
            aTs = []
            wqs = []  # fp8: per k-pair [P, 2*J]; bf16: per k-tile [P, J]

            def emit_weight(kt):
                wr = wraw_pool.tile([P, J], bf16, name=f"wr{kt}", tag="wr")
                nc.sync.dma_start(wr[:], wT_d[kt * P : (kt + 1) * P, :])
                if fp8:
                    if kt % 2 == 0:
                        wqs.append(
                            wq_pool.tile(
                                [P, 2 * J], cdt, name=f"wq{kt // 2}", tag=f"wq{kt // 2}"
                            )
                        )
                    dst = wqs[-1][:, (kt % 2) * J : (kt % 2 + 1) * J]
                    nc.scalar.sign(dst, wr[:])
                else:
                    wq_t = wq_pool.tile([P, J], cdt, name=f"wq{kt}", tag=f"wq{kt}")
                    nc.scalar.sign(wq_t[:], wr[:])
                    if fold_gamma:
                        nc.vector.tensor_scalar_mul(wq_t[:], wq_t[:], igs[kt][:, 0:1])
                    wqs.append(wq_t)

            def emit_act(mt):
                a_raw = act_pool.tile([P, K], f32, name=f"a_raw{mt}", tag="a_raw")
                nc.sync.dma_start(a_raw[:], x_d[mt * P : (mt + 1) * P, :])
                nc.vector.tensor_reduce(
                    scale_all[:, mt : mt + 1],
                    a_raw[:],
                    axis=mybir.AxisListType.X,
                    op=mybir.AluOpType.max,
                    apply_absolute_value=True,
                )
                asg = asg_pool.tile([P, K], bf16, name=f"asg{mt}", tag="asg")
                nc.scalar.sign(asg[:], a_raw[:])
                # PE transpose each [128,128] block (bf16 -- the fp8 transpose
                # path needs strided PSUM writes) into one PSUM tile, then a
                # single DVE copy back to SBUF casts to the matmul dtype.
                # aT layout is k-major planar: block kt at [:, kt*128:+128] --
                # the [p, 2, m] planes DoubleRow wants at pair c = kt//2.
                tp = tpsum_pool.tile([P, K], bf16, name=f"tp{mt}", tag="tp")
                for kt in range(n_kt):
                    nc.tensor.transpose(
                        tp[:, kt * P : (kt + 1) * P],
                        asg[:, kt * P : (kt + 1) * P],
                        ident[:],
                    )
                aT = aT_pool.tile([P, K], cdt, name=f"aT{mt}", tag=f"aT{mt}")
                nc.vector.tensor_copy(aT[:], tp[:])
                aTs.append(aT)

            # Interleaved emission so the single SP DMA ring serves both
            # streams fairly: act tile mt, then weight k-tiles 2mt, 2mt+1.
            k_per_mt = (n_kt + n_mt - 1) // n_mt
            ki = 0
            for mt in range(n_mt):
                emit_act(mt)
                for _ in range(k_per_mt):
                    if ki < n_kt:
                        emit_weight(ki)
                        ki += 1
            while ki < n_kt:
                emit_weight(ki)
                ki += 1

            # ---- matmuls, m-outer, two K passes ----
            # Pass A (first half of K) evicts psum*scale into an SBUF
            # accumulator; pass B (second half) evicts (psum*scale)+accA in
            # one fused DVE op.  Halving the K-depth per PSUM group lets the
            # PE front-run the weight stream twice as far with only 3
            # resident PSUM half-tiles.
            n_half = J // PSJ
            n_groups = n_kt // 2 if fp8 else n_kt  # accumulation steps total

            def emit_mms(mt, g0, g1, start):
                halves = [
                    psum_pool.tile([P, PSJ], f32, name=f"ps{mt}_{g0}_{h}", tag="ps")
                    for h in range(n_half)
                ]
                for g in range(g0, g1):
                    if fp8:
                        lhsT = aTs[mt][:, g * 256 : (g + 1) * 256].rearrange(
                            "p (two m) -> p two m", two=2
                        )
                        rhs3 = wqs[g][:].rearrange("p (two j) -> p two j", two=2)
                    else:
                        lhsT = aTs[mt][:, g * P : (g + 1) * P]
                    for nj in range(n_nj):
                        ph, off = halves[nj // 2], (nj % 2) * NJ
                        if fp8:
                            nc.tensor.matmul(
                                ph[:, off : off + NJ],
                                lhsT=lhsT,
                                rhs=rhs3[:, :, nj * NJ : (nj + 1) * NJ],
                                start=(g == g0),
                                stop=(g == g1 - 1),
                                perf_mode=mybir.MatmulPerfMode.DoubleRow,
                            )
                        else:
                            nc.tensor.matmul(
                                ph[:, off : off + NJ],
                                lhsT=lhsT,
                                rhs=wqs[g][:, nj * NJ : (nj + 1) * NJ],
                                start=(g == g0),
                                stop=(g == g1 - 1),
                            )
                return halves

            for mt in range(n_mt):
                halves = emit_mms(mt, 0, n_groups, True)
                for h in range(n_half):
                    oc = out_pool.tile([P, PSJ], odt, name=f"oc{mt}_{h}", tag="oc")
                    if h % 2:
                        nc.scalar.mul(oc[:], halves[h][:], scale_all[:, mt : mt + 1])
                    else:
                        nc.vector.tensor_scalar_mul(
                            oc[:], halves[h][:], scale_all[:, mt : mt + 1]
                        )
                    if apply_bb:
                        s = slice(h * PSJ, (h + 1) * PSJ)
                        nc.vector.tensor_tensor(
                            oc[:], oc[:], beta_bc[:, s], mybir.AluOpType.mult
                        )
                        nc.vector.tensor_tensor(
                            oc[:], oc[:], bbeta_bc[:, s], mybir.AluOpType.add
                        )
                    nc.sync.dma_start(
                        out_d[mt * P : (mt + 1) * P, h * PSJ : (h + 1) * PSJ], oc[:]
                    )

        if loop_n:
            with tc.For_i(0, loop_n, 1, hint_engines=(mybir.EngineType.PE,)):
                body()
        else:
            body()
    nc.compile()
    return nc


def _host_prep(input, weight, bias, gamma, beta):
    """Choose mode and build per-core inputs. Host work is layout-only: the
    bf16 cast of the weight preserves every sign bit, so the device-side
    sign() sees identical signs and the kernel result is unchanged."""
    import ml_dtypes

    gamma = np.asarray(gamma, np.float32)
    bias = np.asarray(bias, np.float32)
    beta = np.asarray(beta, np.float32)
    input = np.ascontiguousarray(np.asarray(input, np.float32))
    weight = np.asarray(weight, np.float32)

    fold_gamma = not np.all(gamma == 1.0)
    apply_bb = not (np.all(bias == 0.0) and np.all(beta == 1.0))
    mode = "bf16" if fold_gamma else "fp8"

    wT = np.ascontiguousarray(weight.T.astype(ml_dtypes.bfloat16))  # [K, J]
    extras = {}
    if fold_gamma:
        extras["ig"] = np.ascontiguousarray((1.0 / gamma)[:, None])
    if apply_bb:
        extras["bb"] = np.ascontiguousarray(
            np.stack([beta, bias * beta]).astype(np.float32)
        )
    return input, wT, extras, mode, fold_gamma, apply_bb


def make_in_maps(input, wT, extras):
    N = input.shape[0]
    M = N // N_CORES
    return [
        {"x": np.ascontiguousarray(input[c * M : (c + 1) * M]), "wT": wT, **extras}
        for c in range(N_CORES)
    ]


def kernel(input, weight, bias, gamma, beta):
    input, wT, extras, mode, fold_gamma, apply_bb = _host_prep(
        input, weight, bias, gamma, beta
    )
    N, K = input.shape
    J = wT.shape[1]
    assert N % N_CORES == 0
    M = N // N_CORES

    nc = build_program(M, K, J, mode=mode, fold_gamma=fold_gamma, apply_bb=apply_bb)

    from concourse.bass_utils import run_bass_kernel_spmd

    res = run_bass_kernel_spmd(nc, make_in_maps(input, wT, extras), list(range(N_CORES)))
    out = np.concatenate([r["out"] for r in res.results], axis=0)
    return np.ascontiguousarray(out.astype(np.float32))


if __name__ == "__main__":
    x = np.random.randn(16, 512).astype(np.float32)
    print(
        _host_prep(
            x, np.random.randn(256, 512), np.zeros(256), np.ones(512), np.ones(256)
        )[3]
    )



# revision 8
# speedup vs baseline: 6.8870x; 6.8870x over previous
"""BitLinear Trainium2 kernel.

Computes, for input [N, IN_F], weight [OUT_F, IN_F], bias/beta [OUT_F], gamma [IN_F]:
    scale_i = max_k |input[i, k]|                         (per-row quant scale)
    out[i, j] = sum_k sign(input[i,k]) * (scale_i / gamma[k]) * sign(weight[j,k])
    out = (out + bias) * beta

Strategy: data-parallel shard input rows across 8 NeuronCores; every core holds
the full weight.  Host work is layout/dtype-only (transpose + sign-preserving
casts); all reductions/sign/matmul run on device.

Fast path (gamma==1, bias==0, beta==1 -- the shipped problem):
  - host sends xT = x.T as bf16 (sign bits and the per-row max-abs survive the
    bf16 cast to ~0.2%) and wT = w.T as bf16 (sign-exact cast; ACT sign with
    an fp8 INPUT produces NaN on real HW, so the weight stream stays bf16
    until the device-side sign writes the fp8 +-1 plane).
  - device: ACT signs both streams into fp8 +-1 planes; DVE builds the
    per-row abs-max via an abs_max accumulation chain over the k-tiles of xT
    (layout [k, m], so the chain is elementwise) followed by 8 tiny PE
    transposes + per-block max-reduces to land scale on the output row
    partitions; PE runs the sign matmul in fp8 DoubleRow (exact: +-1 operands,
    integer accumulation in fp32 PSUM); eviction applies scale as a
    per-partition scalar multiply and stores bf16 (host upcasts).
  - LDWEIGHTS dedup: the Tile scheduler emits one Ldweights per matmul even
    when consecutive matmuls share the stationary operand; a post-pass drops
    the redundant reloads (verified on HW: walrus pairs each MMUL with the
    nearest preceding LDW, and the PE array retains weights across MMULs).

General path (nonuniform gamma / bias / beta): previous-generation kernel,
bf16 matmul with 1/gamma folded into the quantized weight.
"""

import os
import sys
import numpy as np
from contextlib import ExitStack

sys.path.insert(0, "/opt/trn_rl_repo")

N_FULL, IN_F, OUT_F = 8192, 2048, 2048
N_CORES = 8
P = 128
NJ = 512  # matmul output column chunk (one PSUM bank)
PSJ = 1024  # psum tile width (2 banks)
W8_SCALE = 8192.0  # sign-preserving pre-scale for the fp8 weight cast


def dedup_ldweights(nc):
    """Drop back-to-back InstLdweights with identical weight APs.

    Walrus lowers each InstMatmult's MMUL from the nearest preceding
    InstLdweights and the PE array keeps the stationary operand across
    matmuls, so an identical reload is pure overhead (~220 cycles each in
    DoubleRow).  fp32 matmuls self-load at the ISA level, so skip those.
    """
    from concourse import mybir

    SAFE = (mybir.InstMatmult, mybir.InstEventSemaphore, mybir.InstDrain)
    f32_dtypes = (mybir.dt.float32, mybir.dt.float32r)
    n_removed = 0
    for blk in nc.m.functions[0].blocks:
        insts = blk.instructions
        out = []
        changed = False
        last_key = None
        for inst in insts:
            if isinstance(inst, mybir.InstLdweights):
                ap = inst.ins[0]
                if getattr(ap, "dtype", None) in f32_dtypes:
                    last_key = None
                    out.append(inst)
                    continue
                key = (
                    repr(inst.ins),
                    str(inst.perf_mode),
                    str(inst.is_transpose),
                    str(inst.tile_position),
                    str(inst.tile_size),
                )
                if key == last_key:
                    n_removed += 1
                    changed = True
                    continue
                last_key = key
            elif not isinstance(inst, SAFE):
                last_key = None
            out.append(inst)
        if changed:
            blk.instructions = out
    return n_removed


def build_fast(M, K, J, loop_n=0, unroll_n=0):
    """Single-core Bass program for the gamma==1, no-bias/beta BitLinear shard.

    DRAM inputs: xT [K, M] bf16 (pre-transposed activations), wT [K, J] bf16
    (pre-transposed weight).  Output: out [M, J] bf16.
    """
    import concourse.bass as bass
    import concourse.tile as tile
    from concourse import bacc, mybir
    from concourse.masks import make_identity

    assert M % P == 0 and K % (2 * P) == 0 and J % PSJ == 0
    n_mt, n_kt = M // P, K // P
    n_g = n_kt // 2
    n_half = J // PSJ
    fp8 = mybir.dt.float8e4
    bf16 = mybir.dt.bfloat16
    f32 = mybir.dt.float32

    nc = bacc.Bacc("TRN2", target_bir_lowering=False, debug=False)
    xT_d = nc.dram_tensor("xT", [K, M], bf16, kind="ExternalInput")
    wT_d = nc.dram_tensor("wT", [K, J], bf16, kind="ExternalInput")
    out_d = nc.dram_tensor("out", [M, J], bf16, kind="ExternalOutput")

    with tile.TileContext(nc) as tc, ExitStack() as ctx:
        const_pool = ctx.enter_context(tc.tile_pool(name="const", bufs=1))
        xt_pool = ctx.enter_context(tc.tile_pool(name="xt", bufs=1))
        axt_pool = ctx.enter_context(tc.tile_pool(name="axt", bufs=2))
        wq_pool = ctx.enter_context(tc.tile_pool(name="wq", bufs=2))
        wraw_pool = ctx.enter_context(tc.tile_pool(name="wraw", bufs=3))
        macc_pool = ctx.enter_context(tc.tile_pool(name="macc", bufs=2))
        scale_pool = ctx.enter_context(tc.tile_pool(name="scalep", bufs=2))
        out_pool = ctx.enter_context(tc.tile_pool(name="outp", bufs=3))
        psum_pool = ctx.enter_context(tc.tile_pool(name="psum", bufs=3, space="PSUM"))
        tps_pool = ctx.enter_context(tc.tile_pool(name="tps", bufs=1, space="PSUM"))

        ident = const_pool.tile([P, P], bf16, name="ident")
        make_identity(nc, ident)

        def body():
            XT = xt_pool.tile([P, n_kt * M], bf16, name="XT", tag="XT")
            AXT = axt_pool.tile([P, n_kt * M], fp8, name="AXT", tag="AXT")
            WQ = wq_pool.tile([P, n_kt * J], fp8, name="WQ", tag="WQ")
            macc = macc_pool.tile([P, M], bf16, name="macc", tag="macc")
            scale_all = scale_pool.tile([P, n_mt], f32, name="scale", tag="scale")

            # ---- streams: x k-tile, w k-tile, signs ----
            for kt in range(n_kt):
                xs = XT[:, kt * M : (kt + 1) * M]
                nc.sync.dma_start(xs, xT_d[kt * P : (kt + 1) * P, :])
                wr = wraw_pool.tile([P, J], bf16, name=f"wr{kt}", tag="wr")
                nc.sync.dma_start(wr[:], wT_d[kt * P : (kt + 1) * P, :])
                nc.scalar.sign(AXT[:, kt * M : (kt + 1) * M], xs)
                nc.scalar.sign(WQ[:, kt * J : (kt + 1) * J], wr[:])

            # macc[p, m] = max_kt |XT[p, kt*M + m]| -- one strided reduce over
            # the kt axis (innermost after the view permutation).
            nc.vector.tensor_reduce(
                macc[:],
                XT[:].rearrange("p (kt m) -> p m kt", kt=n_kt),
                axis=mybir.AxisListType.X,
                op=mybir.AluOpType.max,
                apply_absolute_value=True,
            )

            # ---- per-row scale: macc [k-lane, m] -> scale_all [m-part, mt] ----
            tp = tps_pool.tile([P, M], bf16, name="tp", tag="tp")
            for mb in range(n_mt):
                nc.tensor.transpose(
                    tp[:, mb * P : (mb + 1) * P],
                    macc[:, mb * P : (mb + 1) * P],
                    ident[:],
                )
            for mb in range(n_mt):
                nc.vector.tensor_reduce(
                    scale_all[:, mb : mb + 1],
                    tp[:, mb * P : (mb + 1) * P],
                    axis=mybir.AxisListType.X,
                    op=mybir.AluOpType.max,
                )

            # ---- matmuls (m-outer, k-pair accumulation), scaled eviction ----
            for mt in range(n_mt):
                halves = [
                    psum_pool.tile([P, PSJ], f32, name=f"ps{mt}_{h}", tag="ps")
                    for h in range(n_half)
                ]
                for g in range(n_g):
                    lhsT = (
                        AXT[:, 2 * g * M : (2 * g + 2) * M]
                        .rearrange("p (two m) -> p two m", two=2)[
                            :, :, mt * P : (mt + 1) * P
                        ]
                    )
                    rhs3 = WQ[:, 2 * g * J : (2 * g + 2) * J].rearrange(
                        "p (two j) -> p two j", two=2
                    )
                    for nj in range(J // NJ):
                        ph, off = halves[nj // 2], (nj % 2) * NJ
                        nc.tensor.matmul(
                            ph[:, off : off + NJ],
                            lhsT=lhsT,
                            rhs=rhs3[:, :, nj * NJ : (nj + 1) * NJ],
                            start=(g == 0),
                            stop=(g == n_g - 1),
                            perf_mode=mybir.MatmulPerfMode.DoubleRow,
                        )
                oc = out_pool.tile([P, J], bf16, name=f"oc{mt}", tag="oc")
                for h in range(n_half):
                    dst = oc[:, h * PSJ : (h + 1) * PSJ]
                    if h % 2:
                        nc.scalar.mul(dst, halves[h][:], scale_all[:, mt : mt + 1])
                    else:
                        nc.vector.tensor_scalar_mul(
                            dst, halves[h][:], scale_all[:, mt : mt + 1]
                        )
                # outputs ride the ACT HWDGE ring so input prefetch on the SP
                # ring is never head-of-line blocked behind stores.
                nc.scalar.dma_start(out_d[mt * P : (mt + 1) * P, :], oc[:])

        if loop_n:
            with tc.For_i(0, loop_n, 1, hint_engines=(mybir.EngineType.PE,)):
                body()
        elif unroll_n:
            for _ in range(unroll_n):
                body()
        else:
            body()
    dedup_ldweights(nc)
    nc.compile()
    return nc


def build_program(M, K, J, mode="fp8", fold_gamma=False, apply_bb=False, loop_n=0):
    """General-path program (previous generation): x [M, K] f32 row-major,
    wT [K, J] bf16, optional ig [K, 1] f32 and bb [2, J] f32."""
    import concourse.bass as bass
    import concourse.tile as tile
    from concourse import bacc, mybir
    from concourse.masks import make_identity

    assert M % P == 0 and K % P == 0 and J % PSJ == 0
    n_mt, n_kt, n_nj = M // P, K // P, J // NJ
    fp8 = mode == "fp8"
    if fp8:
        assert not fold_gamma and n_kt % 2 == 0
    cdt = mybir.dt.float8e4 if fp8 else mybir.dt.bfloat16
    f32 = mybir.dt.float32
    bf16 = mybir.dt.bfloat16

    nc = bacc.Bacc("TRN2", target_bir_lowering=False, debug=False)
    x_d = nc.dram_tensor("x", [M, K], f32, kind="ExternalInput")
    wT_d = nc.dram_tensor("wT", [K, J], bf16, kind="ExternalInput")
    ig_d = (
        nc.dram_tensor("ig", [K, 1], f32, kind="ExternalInput") if fold_gamma else None
    )
    bb_d = (
        nc.dram_tensor("bb", [2, J], f32, kind="ExternalInput") if apply_bb else None
    )
    odt = f32 if apply_bb else bf16
    out_d = nc.dram_tensor("out", [M, J], odt, kind="ExternalOutput")

    with tile.TileContext(nc) as tc, ExitStack() as ctx:
        aT_pool = ctx.enter_context(tc.tile_pool(name="aT", bufs=1))
        act_pool = ctx.enter_context(tc.tile_pool(name="act", bufs=3))
        asg_pool = ctx.enter_context(tc.tile_pool(name="asg", bufs=2))
        const_pool = ctx.enter_context(tc.tile_pool(name="const", bufs=1))
        scale_pool = ctx.enter_context(tc.tile_pool(name="scalep", bufs=1))
        wraw_pool = ctx.enter_context(tc.tile_pool(name="wraw", bufs=3))
        wq_pool = ctx.enter_context(tc.tile_pool(name="wq", bufs=1))
        out_pool = ctx.enter_context(tc.tile_pool(name="outp", bufs=3))
        psum_pool = ctx.enter_context(tc.tile_pool(name="psum", bufs=3, space="PSUM"))
        tpsum_pool = ctx.enter_context(tc.tile_pool(name="tpsum", bufs=1, space="PSUM"))

        ident = const_pool.tile([P, P], bf16, name="ident")
        make_identity(nc, ident)

        beta_bc = bbeta_bc = None
        if apply_bb:
            beta_bc = const_pool.tile([P, J], f32, name="beta_bc")
            bbeta_bc = const_pool.tile([P, J], f32, name="bbeta_bc")
            nc.sync.dma_start(beta_bc[:], bb_d[0:1, :].broadcast_to([P, J]))
            nc.sync.dma_start(bbeta_bc[:], bb_d[1:2, :].broadcast_to([P, J]))

        igs = []
        if fold_gamma:
            for kt in range(n_kt):
                ig_t = const_pool.tile([P, 1], f32, name=f"ig{kt}", tag=f"ig{kt}")
                nc.sync.dma_start(ig_t[:], ig_d[kt * P : (kt + 1) * P, :])
                igs.append(ig_t)

        def body():
            scale_all = scale_pool.tile([P, n_mt], f32, name="scale_all")
            aTs = []
            wqs = []

            def emit_weight(kt):
                wr = wraw_pool.tile([P, J], bf16, name=f"wr{kt}", tag="wr")
                nc.sync.dma_start(wr[:], wT_d[kt * P : (kt + 1) * P, :])
                if fp8:
                    if kt % 2 == 0:
                        wqs.append(
                            wq_pool.tile(
                                [P, 2 * J], cdt, name=f"wq{kt // 2}", tag=f"wq{kt // 2}"
                            )
                        )
                    dst = wqs[-1][:, (kt % 2) * J : (kt % 2 + 1) * J]
                    nc.scalar.sign(dst, wr[:])
                else:
                    wq_t = wq_pool.tile([P, J], cdt, name=f"wq{kt}", tag=f"wq{kt}")
                    nc.scalar.sign(wq_t[:], wr[:])
                    if fold_gamma:
                        nc.vector.tensor_scalar_mul(wq_t[:], wq_t[:], igs[kt][:, 0:1])
                    wqs.append(wq_t)

            def emit_act(mt):
                a_raw = act_pool.tile([P, K], f32, name=f"a_raw{mt}", tag="a_raw")
                nc.sync.dma_start(a_raw[:], x_d[mt * P : (mt + 1) * P, :])
                nc.vector.tensor_reduce(
                    scale_all[:, mt : mt + 1],
                    a_raw[:],
                    axis=mybir.AxisListType.X,
                    op=mybir.AluOpType.max,
                    apply_absolute_value=True,
                )
                asg = asg_pool.tile([P, K], bf16, name=f"asg{mt}", tag="asg")
                nc.scalar.sign(asg[:], a_raw[:])
                tp = tpsum_pool.tile([P, K], bf16, name=f"tp{mt}", tag="tp")
                for kt in range(n_kt):
                    nc.tensor.transpose(
                        tp[:, kt * P : (kt + 1) * P],
                        asg[:, kt * P : (kt + 1) * P],
                        ident[:],
                    )
                aT = aT_pool.tile([P, K], cdt, name=f"aT{mt}", tag=f"aT{mt}")
                nc.vector.tensor_copy(aT[:], tp[:])
                aTs.append(aT)

            k_per_mt = (n_kt + n_mt - 1) // n_mt
            ki = 0
            for mt in range(n_mt):
                emit_act(mt)
                for _ in range(k_per_mt):
                    if ki < n_kt:
                        emit_weight(ki)
                        ki += 1
            while ki < n_kt:
                emit_weight(ki)
                ki += 1

            n_half = J // PSJ
            n_groups = n_kt // 2 if fp8 else n_kt

            def emit_mms(mt, g0, g1, start):
                halves = [
                    psum_pool.tile([P, PSJ], f32, name=f"ps{mt}_{g0}_{h}", tag="ps")
                    for h in range(n_half)
                ]
                for g in range(g0, g1):
                    if fp8:
                        lhsT = aTs[mt][:, g * 256 : (g + 1) * 256].rearrange(
                            "p (two m) -> p two m", two=2
                        )
                        rhs3 = wqs[g][:].rearrange("p (two j) -> p two j", two=2)
                    else:
                        lhsT = aTs[mt][:, g * P : (g + 1) * P]
                    for nj in range(n_nj):
                        ph, off = halves[nj // 2], (nj % 2) * NJ
                        if fp8:
                            nc.tensor.matmul(
                                ph[:, off : off + NJ],
                                lhsT=lhsT,
                                rhs=rhs3[:, :, nj * NJ : (nj + 1) * NJ],
                                start=(g == g0),
                                stop=(g == g1 - 1),
                                perf_mode=mybir.MatmulPerfMode.DoubleRow,
                            )
                        else:
                            nc.tensor.matmul(
                                ph[:, off : off + NJ],
                                lhsT=lhsT,
                                rhs=wqs[g][:, nj * NJ : (nj + 1) * NJ],
                                start=(g == g0),
                                stop=(g == g1 - 1),
                            )
                return halves

            for mt in range(n_mt):
                halves = emit_mms(mt, 0, n_groups, True)
                for h in range(n_half):
                    oc = out_pool.tile([P, PSJ], odt, name=f"oc{mt}_{h}", tag="oc")
                    if h % 2:
                        nc.scalar.mul(oc[:], halves[h][:], scale_all[:, mt : mt + 1])
                    else:
                        nc.vector.tensor_scalar_mul(
                            oc[:], halves[h][:], scale_all[:, mt : mt + 1]
                        )
                    if apply_bb:
                        s = slice(h * PSJ, (h + 1) * PSJ)
                        nc.vector.tensor_tensor(
                            oc[:], oc[:], beta_bc[:, s], mybir.AluOpType.mult
                        )
                        nc.vector.tensor_tensor(
                            oc[:], oc[:], bbeta_bc[:, s], mybir.AluOpType.add
                        )
                    nc.sync.dma_start(
                        out_d[mt * P : (mt + 1) * P, h * PSJ : (h + 1) * PSJ], oc[:]
                    )

        if loop_n:
            with tc.For_i(0, loop_n, 1, hint_engines=(mybir.EngineType.PE,)):
                body()
        else:
            body()
    dedup_ldweights(nc)
    nc.compile()
    return nc


def _host_prep(input, weight, bias, gamma, beta):
    """Choose path and build per-core inputs.  Host work is layout/dtype-only:
    transposes plus sign-preserving casts; every reduction and sign() runs on
    device."""
    import ml_dtypes

    gamma = np.asarray(gamma, np.float32)
    bias = np.asarray(bias, np.float32)
    beta = np.asarray(beta, np.float32)
    input = np.asarray(input, np.float32)
    weight = np.asarray(weight, np.float32)

    fold_gamma = not np.all(gamma == 1.0)
    apply_bb = not (np.all(bias == 0.0) and np.all(beta == 1.0))

    if not fold_gamma and not apply_bb:
        # fast path
        N = input.shape[0]
        M = N // N_CORES
        x16 = input.astype(ml_dtypes.bfloat16)
        xTs = [
            np.ascontiguousarray(x16[c * M : (c + 1) * M].T) for c in range(N_CORES)
        ]
        wT = np.ascontiguousarray(weight.T.astype(ml_dtypes.bfloat16))
        in_maps = [{"xT": xTs[c], "wT": wT} for c in range(N_CORES)]
        return {"path": "fast", "in_maps": in_maps, "M": M,
                "K": input.shape[1], "J": weight.shape[0]}

    # general path
    input = np.ascontiguousarray(input)
    mode = "bf16" if fold_gamma else "fp8"
    wT = np.ascontiguousarray(weight.T.astype(ml_dtypes.bfloat16))
    extras = {}
    if fold_gamma:
        extras["ig"] = np.ascontiguousarray((1.0 / gamma)[:, None])
    if apply_bb:
        extras["bb"] = np.ascontiguousarray(
            np.stack([beta, bias * beta]).astype(np.float32)
        )
    N = input.shape[0]
    M = N // N_CORES
    in_maps = [
        {"x": np.ascontiguousarray(input[c * M : (c + 1) * M]), "wT": wT, **extras}
        for c in range(N_CORES)
    ]
    return {"path": "general", "in_maps": in_maps, "M": M, "K": input.shape[1],
            "J": weight.shape[0], "mode": mode, "fold_gamma": fold_gamma,
            "apply_bb": apply_bb}


def prep_and_build(input, weight, bias, gamma, beta, loop_n=0):
    """Returns (nc, in_maps). The program's 'out' outputs concatenate to the
    full [N, OUT_F] result (upcast to f32 by the caller)."""
    prep = _host_prep(input, weight, bias, gamma, beta)
    if prep["path"] == "fast":
        nc = build_fast(prep["M"], prep["K"], prep["J"], loop_n=loop_n)
    else:
        nc = build_program(
            prep["M"], prep["K"], prep["J"], mode=prep["mode"],
            fold_gamma=prep["fold_gamma"], apply_bb=prep["apply_bb"],
            loop_n=loop_n,
        )
    return nc, prep["in_maps"]


def kernel(input, weight, bias, gamma, beta):
    nc, in_maps = prep_and_build(input, weight, bias, gamma, beta)

    from concourse.bass_utils import run_bass_kernel_spmd

    res = run_bass_kernel_spmd(nc, in_maps, list(range(N_CORES)))
    out = np.concatenate([r["out"] for r in res.results], axis=0)
    return np.ascontiguousarray(out.astype(np.float32))


if __name__ == "__main__":
    x = np.random.randn(1024, 512).astype(np.float32)
    w = np.random.randn(512, 512).astype(np.float32) * 0.01
    print(_host_prep(x, w, np.zeros(512), np.ones(512), np.ones(512))["path"])


# revision 9
# speedup vs baseline: 8.0297x; 1.1659x over previous
"""BitLinear Trainium2 kernel.

Computes, for input [N, IN_F], weight [OUT_F, IN_F], bias/beta [OUT_F], gamma [IN_F]:
    scale_i = max_k |input[i, k]|                         (per-row quant scale)
    out[i, j] = sum_k sign(input[i,k]) * (scale_i / gamma[k]) * sign(weight[j,k])
    out = (out + bias) * beta

Strategy: data-parallel shard input rows across 8 NeuronCores; every core holds
the full weight.  Host work is layout/dtype-only (transpose + sign-preserving
casts); all reductions/sign/matmul run on device.

Fast path (gamma==1, bias==0, beta==1 -- the shipped problem):
  - host sends xT = x.T as bf16 (sign bits and the per-row max-abs survive the
    bf16 cast to ~0.2%) and wT = w.T as bf16 (sign-exact cast; ACT sign with
    an fp8 INPUT produces NaN on real HW, so the weight stream stays bf16
    until the device-side sign writes the fp8 +-1 plane).
  - device: ACT signs both streams into fp8 +-1 planes; DVE computes the
    per-row abs-max with one strided tensor_reduce over the kt axis of the
    resident xT (layout [k, m]) followed by 8 tiny PE transposes + per-block
    max-reduces to land scale on the output row partitions; PE runs the sign matmul in fp8 DoubleRow (exact: +-1 operands,
    integer accumulation in fp32 PSUM); eviction applies scale as a
    per-partition scalar multiply and stores bf16 (host upcasts).
  - LDWEIGHTS dedup: the Tile scheduler emits one Ldweights per matmul even
    when consecutive matmuls share the stationary operand; a post-pass drops
    the redundant reloads (verified on HW: walrus pairs each MMUL with the
    nearest preceding LDW, and the PE array retains weights across MMULs).

General path (nonuniform gamma / bias / beta): previous-generation kernel,
bf16 matmul with 1/gamma folded into the quantized weight.
"""

import os
import sys
import numpy as np
from contextlib import ExitStack

sys.path.insert(0, "/opt/trn_rl_repo")

N_FULL, IN_F, OUT_F = 8192, 2048, 2048
N_CORES = 8
P = 128
NJ = 512  # matmul output column chunk (one PSUM bank)
PSJ = 1024  # psum tile width (2 banks)


def dedup_ldweights(nc):
    """Drop back-to-back InstLdweights with identical weight APs.

    Walrus lowers each InstMatmult's MMUL from the nearest preceding
    InstLdweights and the PE array keeps the stationary operand across
    matmuls, so an identical reload is pure overhead (~220 cycles each in
    DoubleRow).  fp32 matmuls self-load at the ISA level, so skip those.
    """
    from concourse import mybir

    SAFE = (mybir.InstMatmult, mybir.InstEventSemaphore, mybir.InstDrain)
    f32_dtypes = (mybir.dt.float32, mybir.dt.float32r)
    n_removed = 0
    for blk in nc.m.functions[0].blocks:
        insts = blk.instructions
        out = []
        changed = False
        last_key = None
        for inst in insts:
            if isinstance(inst, mybir.InstLdweights):
                ap = inst.ins[0]
                if getattr(ap, "dtype", None) in f32_dtypes:
                    last_key = None
                    out.append(inst)
                    continue
                key = (
                    repr(inst.ins),
                    str(inst.perf_mode),
                    str(inst.is_transpose),
                    str(inst.tile_position),
                    str(inst.tile_size),
                )
                if key == last_key:
                    n_removed += 1
                    changed = True
                    continue
                last_key = key
            elif not isinstance(inst, SAFE):
                last_key = None
            out.append(inst)
        if changed:
            blk.instructions = out
    return n_removed


def build_fast(M, K, J, loop_n=0, unroll_n=0):
    """Single-core Bass program for the gamma==1, no-bias/beta BitLinear shard.

    DRAM inputs: xT [K, M] bf16 (pre-transposed activations), wT [K, J] bf16
    (pre-transposed weight).  Output: out [M, J] bf16.
    """
    import concourse.bass as bass
    import concourse.tile as tile
    from concourse import bacc, mybir
    from concourse.masks import make_identity

    assert M % P == 0 and K % (2 * P) == 0 and J % PSJ == 0
    n_mt, n_kt = M // P, K // P
    n_g = n_kt // 2
    n_half = J // PSJ
    fp8 = mybir.dt.float8e4
    bf16 = mybir.dt.bfloat16
    f32 = mybir.dt.float32

    nc = bacc.Bacc("TRN2", target_bir_lowering=False, debug=False)
    xT_d = nc.dram_tensor("xT", [K, M], bf16, kind="ExternalInput")
    wT_d = nc.dram_tensor("wT", [K, J], bf16, kind="ExternalInput")
    out_d = nc.dram_tensor("out", [M, J], bf16, kind="ExternalOutput")

    with tile.TileContext(nc) as tc, ExitStack() as ctx:
        const_pool = ctx.enter_context(tc.tile_pool(name="const", bufs=1))
        xt_pool = ctx.enter_context(tc.tile_pool(name="xt", bufs=1))
        axt_pool = ctx.enter_context(tc.tile_pool(name="axt", bufs=2))
        wq_pool = ctx.enter_context(tc.tile_pool(name="wq", bufs=2))
        wraw_pool = ctx.enter_context(tc.tile_pool(name="wraw", bufs=3))
        macc_pool = ctx.enter_context(tc.tile_pool(name="macc", bufs=2))
        scale_pool = ctx.enter_context(tc.tile_pool(name="scalep", bufs=2))
        out_pool = ctx.enter_context(tc.tile_pool(name="outp", bufs=3))
        psum_pool = ctx.enter_context(tc.tile_pool(name="psum", bufs=3, space="PSUM"))
        tps_pool = ctx.enter_context(tc.tile_pool(name="tps", bufs=1, space="PSUM"))

        ident = const_pool.tile([P, P], bf16, name="ident")
        make_identity(nc, ident)

        def body():
            XT = xt_pool.tile([P, n_kt * M], bf16, name="XT", tag="XT")
            AXT = axt_pool.tile([P, n_kt * M], fp8, name="AXT", tag="AXT")
            WQ = wq_pool.tile([P, n_kt * J], fp8, name="WQ", tag="WQ")
            macc = macc_pool.tile([P, M], bf16, name="macc", tag="macc")
            scale_all = scale_pool.tile([P, n_mt], f32, name="scale", tag="scale")

            # ---- streams: x k-tile, w k-tile, signs ----
            for kt in range(n_kt):
                xs = XT[:, kt * M : (kt + 1) * M]
                nc.sync.dma_start(xs, xT_d[kt * P : (kt + 1) * P, :])
                wr = wraw_pool.tile([P, J], bf16, name=f"wr{kt}", tag="wr")
                nc.sync.dma_start(wr[:], wT_d[kt * P : (kt + 1) * P, :])
                nc.scalar.sign(AXT[:, kt * M : (kt + 1) * M], xs)
                nc.scalar.sign(WQ[:, kt * J : (kt + 1) * J], wr[:])

            # macc[p, m] = max_kt |XT[p, kt*M + m]| -- one strided reduce over
            # the kt axis (innermost after the view permutation).
            nc.vector.tensor_reduce(
                macc[:],
                XT[:].rearrange("p (kt m) -> p m kt", kt=n_kt),
                axis=mybir.AxisListType.X,
                op=mybir.AluOpType.max,
                apply_absolute_value=True,
            )

            # ---- per-row scale: macc [k-lane, m] -> scale_all [m-part, mt] ----
            tp = tps_pool.tile([P, M], bf16, name="tp", tag="tp")
            for mb in range(n_mt):
                nc.tensor.transpose(
                    tp[:, mb * P : (mb + 1) * P],
                    macc[:, mb * P : (mb + 1) * P],
                    ident[:],
                )
            for mb in range(n_mt):
                nc.vector.tensor_reduce(
                    scale_all[:, mb : mb + 1],
                    tp[:, mb * P : (mb + 1) * P],
                    axis=mybir.AxisListType.X,
                    op=mybir.AluOpType.max,
                )

            # ---- matmuls (m-outer, k-pair accumulation), scaled eviction ----
            for mt in range(n_mt):
                halves = [
                    psum_pool.tile([P, PSJ], f32, name=f"ps{mt}_{h}", tag="ps")
                    for h in range(n_half)
                ]
                for g in range(n_g):
                    lhsT = (
                        AXT[:, 2 * g * M : (2 * g + 2) * M]
                        .rearrange("p (two m) -> p two m", two=2)[
                            :, :, mt * P : (mt + 1) * P
                        ]
                    )
                    rhs3 = WQ[:, 2 * g * J : (2 * g + 2) * J].rearrange(
                        "p (two j) -> p two j", two=2
                    )
                    for nj in range(J // NJ):
                        ph, off = halves[nj // 2], (nj % 2) * NJ
                        nc.tensor.matmul(
                            ph[:, off : off + NJ],
                            lhsT=lhsT,
                            rhs=rhs3[:, :, nj * NJ : (nj + 1) * NJ],
                            start=(g == 0),
                            stop=(g == n_g - 1),
                            perf_mode=mybir.MatmulPerfMode.DoubleRow,
                        )
                oc = out_pool.tile([P, J], bf16, name=f"oc{mt}", tag="oc")
                for h in range(n_half):
                    dst = oc[:, h * PSJ : (h + 1) * PSJ]
                    if h % 2:
                        nc.scalar.mul(dst, halves[h][:], scale_all[:, mt : mt + 1])
                    else:
                        nc.vector.tensor_scalar_mul(
                            dst, halves[h][:], scale_all[:, mt : mt + 1]
                        )
                # outputs ride the ACT HWDGE ring so input prefetch on the SP
                # ring is never head-of-line blocked behind stores.
                nc.scalar.dma_start(out_d[mt * P : (mt + 1) * P, :], oc[:])

        if loop_n:
            with tc.For_i(0, loop_n, 1, hint_engines=(mybir.EngineType.PE,)):
                body()
        elif unroll_n:
            for _ in range(unroll_n):
                body()
        else:
            body()
    dedup_ldweights(nc)
    nc.compile()
    return nc


def build_program(M, K, J, mode="fp8", fold_gamma=False, apply_bb=False, loop_n=0):
    """General-path program (previous generation): x [M, K] f32 row-major,
    wT [K, J] bf16, optional ig [K, 1] f32 and bb [2, J] f32."""
    import concourse.bass as bass
    import concourse.tile as tile
    from concourse import bacc, mybir
    from concourse.masks import make_identity

    assert M % P == 0 and K % P == 0 and J % PSJ == 0
    n_mt, n_kt, n_nj = M // P, K // P, J // NJ
    fp8 = mode == "fp8"
    if fp8:
        assert not fold_gamma and n_kt % 2 == 0
    cdt = mybir.dt.float8e4 if fp8 else mybir.dt.bfloat16
    f32 = mybir.dt.float32
    bf16 = mybir.dt.bfloat16

    nc = bacc.Bacc("TRN2", target_bir_lowering=False, debug=False)
    x_d = nc.dram_tensor("x", [M, K], f32, kind="ExternalInput")
    wT_d = nc.dram_tensor("wT", [K, J], bf16, kind="ExternalInput")
    ig_d = (
        nc.dram_tensor("ig", [K, 1], f32, kind="ExternalInput") if fold_gamma else None
    )
    bb_d = (
        nc.dram_tensor("bb", [2, J], f32, kind="ExternalInput") if apply_bb else None
    )
    odt = f32 if apply_bb else bf16
    out_d = nc.dram_tensor("out", [M, J], odt, kind="ExternalOutput")

    with tile.TileContext(nc) as tc, ExitStack() as ctx:
        aT_pool = ctx.enter_context(tc.tile_pool(name="aT", bufs=1))
        act_pool = ctx.enter_context(tc.tile_pool(name="act", bufs=3))
        asg_pool = ctx.enter_context(tc.tile_pool(name="asg", bufs=2))
        const_pool = ctx.enter_context(tc.tile_pool(name="const", bufs=1))
        scale_pool = ctx.enter_context(tc.tile_pool(name="scalep", bufs=1))
        wraw_pool = ctx.enter_context(tc.tile_pool(name="wraw", bufs=3))
        wq_pool = ctx.enter_context(tc.tile_pool(name="wq", bufs=1))
        out_pool = ctx.enter_context(tc.tile_pool(name="outp", bufs=3))
        psum_pool = ctx.enter_context(tc.tile_pool(name="psum", bufs=3, space="PSUM"))
        tpsum_pool = ctx.enter_context(tc.tile_pool(name="tpsum", bufs=1, space="PSUM"))

        ident = const_pool.tile([P, P], bf16, name="ident")
        make_identity(nc, ident)

        beta_bc = bbeta_bc = None
        if apply_bb:
            beta_bc = const_pool.tile([P, J], f32, name="beta_bc")
            bbeta_bc = const_pool.tile([P, J], f32, name="bbeta_bc")
            nc.sync.dma_start(beta_bc[:], bb_d[0:1, :].broadcast_to([P, J]))
            nc.sync.dma_start(bbeta_bc[:], bb_d[1:2, :].broadcast_to([P, J]))

        igs = []
        if fold_gamma:
            for kt in range(n_kt):
                ig_t = const_pool.tile([P, 1], f32, name=f"ig{kt}", tag=f"ig{kt}")
                nc.sync.dma_start(ig_t[:], ig_d[kt * P : (kt + 1) * P, :])
                igs.append(ig_t)

        def body():
            scale_all = scale_pool.tile([P, n_mt], f32, name="scale_all")
            aTs = []
            wqs = []

            def emit_weight(kt):
                wr = wraw_pool.tile([P, J], bf16, name=f"wr{kt}", tag="wr")
                nc.sync.dma_start(wr[:], wT_d[kt * P : (kt + 1) * P, :])
                if fp8:
                    if kt % 2 == 0:
                        wqs.append(
                            wq_pool.tile(
                                [P, 2 * J], cdt, name=f"wq{kt // 2}", tag=f"wq{kt // 2}"
                            )
                        )
                    dst = wqs[-1][:, (kt % 2) * J : (kt % 2 + 1) * J]
                    nc.scalar.sign(dst, wr[:])
                else:
                    wq_t = wq_pool.tile([P, J], cdt, name=f"wq{kt}", tag=f"wq{kt}")
                    nc.scalar.sign(wq_t[:], wr[:])
                    if fold_gamma:
                        nc.vector.tensor_scalar_mul(wq_t[:], wq_t[:], igs[kt][:, 0:1])
                    wqs.append(wq_t)

            def emit_act(mt):
                a_raw = act_pool.tile([P, K], f32, name=f"a_raw{mt}", tag="a_raw")
                nc.sync.dma_start(a_raw[:], x_d[mt * P : (mt + 1) * P, :])
                nc.vector.tensor_reduce(
                    scale_all[:, mt : mt + 1],
                    a_raw[:],
                    axis=mybir.AxisListType.X,
                    op=mybir.AluOpType.max,
                    apply_absolute_value=True,
                )
                asg = asg_pool.tile([P, K], bf16, name=f"asg{mt}", tag="asg")
                nc.scalar.sign(asg[:], a_raw[:])
                tp = tpsum_pool.tile([P, K], bf16, name=f"tp{mt}", tag="tp")
                for kt in range(n_kt):
                    nc.tensor.transpose(
                        tp[:, kt * P : (kt + 1) * P],
                        asg[:, kt * P : (kt + 1) * P],
                        ident[:],
                    )
                aT = aT_pool.tile([P, K], cdt, name=f"aT{mt}", tag=f"aT{mt}")
                nc.vector.tensor_copy(aT[:], tp[:])
                aTs.append(aT)

            k_per_mt = (n_kt + n_mt - 1) // n_mt
            ki = 0
            for mt in range(n_mt):
                emit_act(mt)
                for _ in range(k_per_mt):
                    if ki < n_kt:
                        emit_weight(ki)
                        ki += 1
            while ki < n_kt:
                emit_weight(ki)
                ki += 1

            n_half = J // PSJ
            n_groups = n_kt // 2 if fp8 else n_kt

            def emit_mms(mt, g0, g1, start):
                halves = [
                    psum_pool.tile([P, PSJ], f32, name=f"ps{mt}_{g0}_{h}", tag="ps")
                    for h in range(n_half)
                ]
                for g in range(g0, g1):
                    if fp8:
                        lhsT = aTs[mt][:, g * 256 : (g + 1) * 256].rearrange(
                            "p (two m) -> p two m", two=2
                        )
                        rhs3 = wqs[g][:].rearrange("p (two j) -> p two j", two=2)
                    else:
                        lhsT = aTs[mt][:, g * P : (g + 1) * P]
                    for nj in range(n_nj):
                        ph, off = halves[nj // 2], (nj % 2) * NJ
                        if fp8:
                            nc.tensor.matmul(
                                ph[:, off : off + NJ],
                                lhsT=lhsT,
                                rhs=rhs3[:, :, nj * NJ : (nj + 1) * NJ],
                                start=(g == g0),
                                stop=(g == g1 - 1),
                                perf_mode=mybir.MatmulPerfMode.DoubleRow,
                            )
                        else:
                            nc.tensor.matmul(
                                ph[:, off : off + NJ],
                                lhsT=lhsT,
                                rhs=wqs[g][:, nj * NJ : (nj + 1) * NJ],
                                start=(g == g0),
                                stop=(g == g1 - 1),
                            )
                return halves

            for mt in range(n_mt):
                halves = emit_mms(mt, 0, n_groups, True)
                for h in range(n_half):
                    oc = out_pool.tile([P, PSJ], odt, name=f"oc{mt}_{h}", tag="oc")
                    if h % 2:
                        nc.scalar.mul(oc[:], halves[h][:], scale_all[:, mt : mt + 1])
                    else:
                        nc.vector.tensor_scalar_mul(
                            oc[:], halves[h][:], scale_all[:, mt : mt + 1]
                        )
                    if apply_bb:
                        s = slice(h * PSJ, (h + 1) * PSJ)
                        nc.vector.tensor_tensor(
                            oc[:], oc[:], beta_bc[:, s], mybir.AluOpType.mult
                        )
                        nc.vector.tensor_tensor(
                            oc[:], oc[:], bbeta_bc[:, s], mybir.AluOpType.add
                        )
                    nc.sync.dma_start(
                        out_d[mt * P : (mt + 1) * P, h * PSJ : (h + 1) * PSJ], oc[:]
                    )

        if loop_n:
            with tc.For_i(0, loop_n, 1, hint_engines=(mybir.EngineType.PE,)):
                body()
        else:
            body()
    dedup_ldweights(nc)
    nc.compile()
    return nc


def _host_prep(input, weight, bias, gamma, beta):
    """Choose path and build per-core inputs.  Host work is layout/dtype-only:
    transposes plus sign-preserving casts; every reduction and sign() runs on
    device."""
    import ml_dtypes

    gamma = np.asarray(gamma, np.float32)
    bias = np.asarray(bias, np.float32)
    beta = np.asarray(beta, np.float32)
    input = np.asarray(input, np.float32)
    weight = np.asarray(weight, np.float32)

    fold_gamma = not np.all(gamma == 1.0)
    apply_bb = not (np.all(bias == 0.0) and np.all(beta == 1.0))

    if not fold_gamma and not apply_bb:
        # fast path
        N = input.shape[0]
        M = N // N_CORES
        x16 = input.astype(ml_dtypes.bfloat16)
        xTs = [
            np.ascontiguousarray(x16[c * M : (c + 1) * M].T) for c in range(N_CORES)
        ]
        wT = np.ascontiguousarray(weight.T.astype(ml_dtypes.bfloat16))
        in_maps = [{"xT": xTs[c], "wT": wT} for c in range(N_CORES)]
        return {"path": "fast", "in_maps": in_maps, "M": M,
                "K": input.shape[1], "J": weight.shape[0]}

    # general path
    input = np.ascontiguousarray(input)
    mode = "bf16" if fold_gamma else "fp8"
    wT = np.ascontiguousarray(weight.T.astype(ml_dtypes.bfloat16))
    extras = {}
    if fold_gamma:
        extras["ig"] = np.ascontiguousarray((1.0 / gamma)[:, None])
    if apply_bb:
        extras["bb"] = np.ascontiguousarray(
            np.stack([beta, bias * beta]).astype(np.float32)
        )
    N = input.shape[0]
    M = N // N_CORES
    in_maps = [
        {"x": np.ascontiguousarray(input[c * M : (c + 1) * M]), "wT": wT, **extras}
        for c in range(N_CORES)
    ]
    return {"path": "general", "in_maps": in_maps, "M": M, "K": input.shape[1],
            "J": weight.shape[0], "mode": mode, "fold_gamma": fold_gamma,
            "apply_bb": apply_bb}


def prep_and_build(input, weight, bias, gamma, beta, loop_n=0):
    """Returns (nc, in_maps). The program's 'out' outputs concatenate to the
    full [N, OUT_F] result (upcast to f32 by the caller)."""
    prep = _host_prep(input, weight, bias, gamma, beta)
    if prep["path"] == "fast":
        nc = build_fast(prep["M"], prep["K"], prep["J"], loop_n=loop_n)
    else:
        nc = build_program(
            prep["M"], prep["K"], prep["J"], mode=prep["mode"],
            fold_gamma=prep["fold_gamma"], apply_bb=prep["apply_bb"],
            loop_n=loop_n,
        )
    return nc, prep["in_maps"]


def kernel(input, weight, bias, gamma, beta):
    nc, in_maps = prep_and_build(input, weight, bias, gamma, beta)

    from concourse.bass_utils import run_bass_kernel_spmd

    res = run_bass_kernel_spmd(nc, in_maps, list(range(N_CORES)))
    out = np.concatenate([r["out"] for r in res.results], axis=0)
    return np.ascontiguousarray(out.astype(np.float32))


if __name__ == "__main__":
    x = np.random.randn(1024, 512).astype(np.float32)
    w = np.random.randn(512, 512).astype(np.float32) * 0.01
    print(_host_prep(x, w, np.zeros(512), np.ones(512), np.ones(512))["path"])


# revision 11
# speedup vs baseline: 11.1451x; 1.3880x over previous
"""BitLinear Trainium2 kernel.

Computes, for input [N, IN_F], weight [OUT_F, IN_F], bias/beta [OUT_F], gamma [IN_F]:
    scale_i = max_k |input[i, k]|                         (per-row quant scale)
    out[i, j] = sum_k sign(input[i,k]) * (scale_i / gamma[k]) * sign(weight[j,k])
    out = (out + bias) * beta

Strategy: data-parallel shard input rows across 8 NeuronCores; every core holds
the full weight.  Host work is layout/dtype-only (transpose + sign-preserving
casts); all reductions/sign/matmul run on device.

Fast path (gamma==1, bias==0, beta==1 -- the shipped problem):
  - host sends xT = x.T as bf16 (sign bits and the per-row max-abs survive the
    bf16 cast to ~0.2%) and wT = w.T as bf16 (sign-exact cast; ACT sign with
    an fp8 INPUT produces NaN on real HW, so the weight stream stays bf16
    until the device-side sign writes the fp8 +-1 plane).
  - device: ACT signs both streams into fp8 +-1 planes; DVE computes the
    per-row abs-max with one strided tensor_reduce over the kt axis of the
    resident xT (layout [k, m]) followed by 8 tiny PE transposes + per-block
    max-reduces to land scale on the output row partitions; PE runs the sign matmul in fp8 DoubleRow (exact: +-1 operands,
    integer accumulation in fp32 PSUM); eviction applies scale as a
    per-partition scalar multiply and stores bf16 (host upcasts).
  - LDWEIGHTS dedup: the Tile scheduler emits one Ldweights per matmul even
    when consecutive matmuls share the stationary operand; a post-pass drops
    the redundant reloads (verified on HW: walrus pairs each MMUL with the
    nearest preceding LDW, and the PE array retains weights across MMULs).

General path (nonuniform gamma / bias / beta): previous-generation kernel,
bf16 matmul with 1/gamma folded into the quantized weight.
"""

import os
import sys
import numpy as np
from contextlib import ExitStack

sys.path.insert(0, "/opt/trn_rl_repo")

N_FULL, IN_F, OUT_F = 8192, 2048, 2048
N_CORES = 8
P = 128
NJ = 512  # matmul output column chunk (one PSUM bank)
PSJ = 1024  # psum tile width (2 banks)


def dedup_ldweights(nc):
    """Drop back-to-back InstLdweights with identical weight APs.

    Walrus lowers each InstMatmult's MMUL from the nearest preceding
    InstLdweights and the PE array keeps the stationary operand across
    matmuls, so an identical reload is pure overhead (~220 cycles each in
    DoubleRow).  fp32 matmuls self-load at the ISA level, so skip those.
    """
    from concourse import mybir

    SAFE = (mybir.InstMatmult, mybir.InstEventSemaphore, mybir.InstDrain)
    f32_dtypes = (mybir.dt.float32, mybir.dt.float32r)
    n_removed = 0
    for blk in nc.m.functions[0].blocks:
        insts = blk.instructions
        out = []
        changed = False
        last_key = None
        for inst in insts:
            if isinstance(inst, mybir.InstLdweights):
                ap = inst.ins[0]
                if getattr(ap, "dtype", None) in f32_dtypes:
                    last_key = None
                    out.append(inst)
                    continue
                key = (
                    repr(inst.ins),
                    str(inst.perf_mode),
                    str(inst.is_transpose),
                    str(inst.tile_position),
                    str(inst.tile_size),
                )
                if key == last_key:
                    n_removed += 1
                    changed = True
                    continue
                last_key = key
            elif not isinstance(inst, SAFE):
                last_key = None
            out.append(inst)
        if changed:
            blk.instructions = out
    return n_removed


def build_fast(M, K, J, loop_n=0, unroll_n=0):
    """Single-core Bass program for the gamma==1, no-bias/beta BitLinear shard.

    DRAM inputs: xT [K, M] bf16 (pre-transposed activations), wT [K, J] bf16
    (pre-transposed weight).  Output: out [M, J] bf16.
    """
    import concourse.bass as bass
    import concourse.tile as tile
    from concourse import bacc, mybir
    from concourse.masks import make_identity

    assert M % P == 0 and K % (2 * P) == 0 and J % PSJ == 0
    n_mt, n_kt = M // P, K // P
    n_g = n_kt // 2
    n_half = J // PSJ
    fp8 = mybir.dt.float8e4
    bf16 = mybir.dt.bfloat16
    f32 = mybir.dt.float32

    nc = bacc.Bacc("TRN2", target_bir_lowering=False, debug=False)
    xT_d = nc.dram_tensor("xT", [K, M], bf16, kind="ExternalInput")
    wT_d = nc.dram_tensor("wT", [K, J], bf16, kind="ExternalInput")
    out_d = nc.dram_tensor("out", [M, J], bf16, kind="ExternalOutput")

    with tile.TileContext(nc) as tc, ExitStack() as ctx:
        const_pool = ctx.enter_context(tc.tile_pool(name="const", bufs=1))
        xt_pool = ctx.enter_context(tc.tile_pool(name="xt", bufs=1))
        axt_pool = ctx.enter_context(tc.tile_pool(name="axt", bufs=2))
        wq_pool = ctx.enter_context(tc.tile_pool(name="wq", bufs=2))
        wraw_pool = ctx.enter_context(tc.tile_pool(name="wraw", bufs=3))
        macc_pool = ctx.enter_context(tc.tile_pool(name="macc", bufs=2))
        scale_pool = ctx.enter_context(tc.tile_pool(name="scalep", bufs=2))
        out_pool = ctx.enter_context(tc.tile_pool(name="outp", bufs=3))
        psum_pool = ctx.enter_context(tc.tile_pool(name="psum", bufs=3, space="PSUM"))
        tps_pool = ctx.enter_context(tc.tile_pool(name="tps", bufs=1, space="PSUM"))

        ident = const_pool.tile([P, P], bf16, name="ident")
        make_identity(nc, ident)

        def body():
            XT = xt_pool.tile([P, n_kt * M], bf16, name="XT", tag="XT")
            AXT = axt_pool.tile([P, n_kt * M], fp8, name="AXT", tag="AXT")
            WQ = wq_pool.tile([P, n_kt * J], fp8, name="WQ", tag="WQ")
            macc = macc_pool.tile([P, M], bf16, name="macc", tag="macc")
            scale_all = scale_pool.tile([P, n_mt], f32, name="scale", tag="scale")

            # ---- streams: x k-tile, w k-tile, signs ----
            for kt in range(n_kt):
                xs = XT[:, kt * M : (kt + 1) * M]
                nc.sync.dma_start(xs, xT_d[kt * P : (kt + 1) * P, :])
                wr = wraw_pool.tile([P, J], bf16, name=f"wr{kt}", tag="wr")
                nc.sync.dma_start(wr[:], wT_d[kt * P : (kt + 1) * P, :])
                nc.scalar.sign(AXT[:, kt * M : (kt + 1) * M], xs)
                nc.scalar.sign(WQ[:, kt * J : (kt + 1) * J], wr[:])

            # macc[p, m] = max_kt |XT[p, kt*M + m]| -- one strided reduce over
            # the kt axis (innermost after the view permutation).
            nc.vector.tensor_reduce(
                macc[:],
                XT[:].rearrange("p (kt m) -> p m kt", kt=n_kt),
                axis=mybir.AxisListType.X,
                op=mybir.AluOpType.max,
                apply_absolute_value=True,
            )

            # ---- per-row scale: macc [k-lane, m] -> scale_all [m-part, mt] ----
            tp = tps_pool.tile([P, M], bf16, name="tp", tag="tp")
            for mb in range(n_mt):
                nc.tensor.transpose(
                    tp[:, mb * P : (mb + 1) * P],
                    macc[:, mb * P : (mb + 1) * P],
                    ident[:],
                )
            for mb in range(n_mt):
                nc.vector.tensor_reduce(
                    scale_all[:, mb : mb + 1],
                    tp[:, mb * P : (mb + 1) * P],
                    axis=mybir.AxisListType.X,
                    op=mybir.AluOpType.max,
                )

            # ---- matmuls (m-outer, k-pair accumulation), scaled eviction ----
            for mt in range(n_mt):
                halves = [
                    psum_pool.tile([P, PSJ], f32, name=f"ps{mt}_{h}", tag="ps")
                    for h in range(n_half)
                ]
                for g in range(n_g):
                    lhsT = (
                        AXT[:, 2 * g * M : (2 * g + 2) * M]
                        .rearrange("p (two m) -> p two m", two=2)[
                            :, :, mt * P : (mt + 1) * P
                        ]
                    )
                    rhs3 = WQ[:, 2 * g * J : (2 * g + 2) * J].rearrange(
                        "p (two j) -> p two j", two=2
                    )
                    for nj in range(J // NJ):
                        ph, off = halves[nj // 2], (nj % 2) * NJ
                        nc.tensor.matmul(
                            ph[:, off : off + NJ],
                            lhsT=lhsT,
                            rhs=rhs3[:, :, nj * NJ : (nj + 1) * NJ],
                            start=(g == 0),
                            stop=(g == n_g - 1),
                            perf_mode=mybir.MatmulPerfMode.DoubleRow,
                        )
                oc = out_pool.tile([P, J], bf16, name=f"oc{mt}", tag="oc")
                for h in range(n_half):
                    dst = oc[:, h * PSJ : (h + 1) * PSJ]
                    if h % 2:
                        nc.scalar.mul(dst, halves[h][:], scale_all[:, mt : mt + 1])
                    else:
                        nc.vector.tensor_scalar_mul(
                            dst, halves[h][:], scale_all[:, mt : mt + 1]
                        )
                # outputs ride the ACT HWDGE ring so input prefetch on the SP
                # ring is never head-of-line blocked behind stores.
                nc.scalar.dma_start(out_d[mt * P : (mt + 1) * P, :], oc[:])

        if loop_n:
            # unroll_n bodies per hardware-loop iteration: consecutive bodies
            # pipeline through the rotating tile pools, so the per-iteration
            # scheduling barrier amortizes over unroll_n kernel executions.
            with tc.For_i(0, loop_n, 1, hint_engines=(mybir.EngineType.PE,)):
                for _ in range(max(1, unroll_n)):
                    body()
        elif unroll_n:
            for _ in range(unroll_n):
                body()
        else:
            body()
    dedup_ldweights(nc)
    nc.compile()
    return nc


def build_program(M, K, J, mode="fp8", fold_gamma=False, apply_bb=False, loop_n=0):
    """General-path program (previous generation): x [M, K] f32 row-major,
    wT [K, J] bf16, optional ig [K, 1] f32 and bb [2, J] f32."""
    import concourse.bass as bass
    import concourse.tile as tile
    from concourse import bacc, mybir
    from concourse.masks import make_identity

    assert M % P == 0 and K % P == 0 and J % PSJ == 0
    n_mt, n_kt, n_nj = M // P, K // P, J // NJ
    fp8 = mode == "fp8"
    if fp8:
        assert not fold_gamma and n_kt % 2 == 0
    cdt = mybir.dt.float8e4 if fp8 else mybir.dt.bfloat16
    f32 = mybir.dt.float32
    bf16 = mybir.dt.bfloat16

    nc = bacc.Bacc("TRN2", target_bir_lowering=False, debug=False)
    x_d = nc.dram_tensor("x", [M, K], f32, kind="ExternalInput")
    wT_d = nc.dram_tensor("wT", [K, J], bf16, kind="ExternalInput")
    ig_d = (
        nc.dram_tensor("ig", [K, 1], f32, kind="ExternalInput") if fold_gamma else None
    )
    bb_d = (
        nc.dram_tensor("bb", [2, J], f32, kind="ExternalInput") if apply_bb else None
    )
    odt = f32 if apply_bb else bf16
    out_d = nc.dram_tensor("out", [M, J], odt, kind="ExternalOutput")

    with tile.TileContext(nc) as tc, ExitStack() as ctx:
        aT_pool = ctx.enter_context(tc.tile_pool(name="aT", bufs=1))
        act_pool = ctx.enter_context(tc.tile_pool(name="act", bufs=3))
        asg_pool = ctx.enter_context(tc.tile_pool(name="asg", bufs=2))
        const_pool = ctx.enter_context(tc.tile_pool(name="const", bufs=1))
        scale_pool = ctx.enter_context(tc.tile_pool(name="scalep", bufs=1))
        wraw_pool = ctx.enter_context(tc.tile_pool(name="wraw", bufs=3))
        wq_pool = ctx.enter_context(tc.tile_pool(name="wq", bufs=1))
        out_pool = ctx.enter_context(tc.tile_pool(name="outp", bufs=3))
        psum_pool = ctx.enter_context(tc.tile_pool(name="psum", bufs=3, space="PSUM"))
        tpsum_pool = ctx.enter_context(tc.tile_pool(name="tpsum", bufs=1, space="PSUM"))

        ident = const_pool.tile([P, P], bf16, name="ident")
        make_identity(nc, ident)

        beta_bc = bbeta_bc = None
        if apply_bb:
            beta_bc = const_pool.tile([P, J], f32, name="beta_bc")
            bbeta_bc = const_pool.tile([P, J], f32, name="bbeta_bc")
            nc.sync.dma_start(beta_bc[:], bb_d[0:1, :].broadcast_to([P, J]))
            nc.sync.dma_start(bbeta_bc[:], bb_d[1:2, :].broadcast_to([P, J]))

        igs = []
        if fold_gamma:
            for kt in range(n_kt):
                ig_t = const_pool.tile([P, 1], f32, name=f"ig{kt}", tag=f"ig{kt}")
                nc.sync.dma_start(ig_t[:], ig_d[kt * P : (kt + 1) * P, :])
                igs.append(ig_t)

        def body():
            scale_all = scale_pool.tile([P, n_mt], f32, name="scale_all")
            aTs = []
            wqs = []

            def emit_weight(kt):
                wr = wraw_pool.tile([P, J], bf16, name=f"wr{kt}", tag="wr")
                nc.sync.dma_start(wr[:], wT_d[kt * P : (kt + 1) * P, :])
                if fp8:
                    if kt % 2 == 0:
                        wqs.append(
                            wq_pool.tile(
                                [P, 2 * J], cdt, name=f"wq{kt // 2}", tag=f"wq{kt // 2}"
                            )
                        )
                    dst = wqs[-1][:, (kt % 2) * J : (kt % 2 + 1) * J]
                    nc.scalar.sign(dst, wr[:])
                else:
                    wq_t = wq_pool.tile([P, J], cdt, name=f"wq{kt}", tag=f"wq{kt}")
                    nc.scalar.sign(wq_t[:], wr[:])
                    if fold_gamma:
                        nc.vector.tensor_scalar_mul(wq_t[:], wq_t[:], igs[kt][:, 0:1])
                    wqs.append(wq_t)

            def emit_act(mt):
                a_raw = act_pool.tile([P, K], f32, name=f"a_raw{mt}", tag="a_raw")
                nc.sync.dma_start(a_raw[:], x_d[mt * P : (mt + 1) * P, :])
                nc.vector.tensor_reduce(
                    scale_all[:, mt : mt + 1],
                    a_raw[:],
                    axis=mybir.AxisListType.X,
                    op=mybir.AluOpType.max,
                    apply_absolute_value=True,
                )
                asg = asg_pool.tile([P, K], bf16, name=f"asg{mt}", tag="asg")
                nc.scalar.sign(asg[:], a_raw[:])
                tp = tpsum_pool.tile([P, K], bf16, name=f"tp{mt}", tag="tp")
                for kt in range(n_kt):
                    nc.tensor.transpose(
                        tp[:, kt * P : (kt + 1) * P],
                        asg[:, kt * P : (kt + 1) * P],
                        ident[:],
                    )
                aT = aT_pool.tile([P, K], cdt, name=f"aT{mt}", tag=f"aT{mt}")
                nc.vector.tensor_copy(aT[:], tp[:])
                aTs.append(aT)

            k_per_mt = (n_kt + n_mt - 1) // n_mt
            ki = 0
            for mt in range(n_mt):
                emit_act(mt)
                for _ in range(k_per_mt):
                    if ki < n_kt:
                        emit_weight(ki)
                        ki += 1
            while ki < n_kt:
                emit_weight(ki)
                ki += 1

            n_half = J // PSJ
            n_groups = n_kt // 2 if fp8 else n_kt

            def emit_mms(mt, g0, g1, start):
                halves = [
                    psum_pool.tile([P, PSJ], f32, name=f"ps{mt}_{g0}_{h}", tag="ps")
                    for h in range(n_half)
                ]
                for g in range(g0, g1):
                    if fp8:
                        lhsT = aTs[mt][:, g * 256 : (g + 1) * 256].rearrange(
                            "p (two m) -> p two m", two=2
                        )
                        rhs3 = wqs[g][:].rearrange("p (two j) -> p two j", two=2)
                    else:
                        lhsT = aTs[mt][:, g * P : (g + 1) * P]
                    for nj in range(n_nj):
                        ph, off = halves[nj // 2], (nj % 2) * NJ
                        if fp8:
                            nc.tensor.matmul(
                                ph[:, off : off + NJ],
                                lhsT=lhsT,
                                rhs=rhs3[:, :, nj * NJ : (nj + 1) * NJ],
                                start=(g == g0),
                                stop=(g == g1 - 1),
                                perf_mode=mybir.MatmulPerfMode.DoubleRow,
                            )
                        else:
                            nc.tensor.matmul(
                                ph[:, off : off + NJ],
                                lhsT=lhsT,
                                rhs=wqs[g][:, nj * NJ : (nj + 1) * NJ],
                                start=(g == g0),
                                stop=(g == g1 - 1),
                            )
                return halves

            for mt in range(n_mt):
                halves = emit_mms(mt, 0, n_groups, True)
                for h in range(n_half):
                    oc = out_pool.tile([P, PSJ], odt, name=f"oc{mt}_{h}", tag="oc")
                    if h % 2:
                        nc.scalar.mul(oc[:], halves[h][:], scale_all[:, mt : mt + 1])
                    else:
                        nc.vector.tensor_scalar_mul(
                            oc[:], halves[h][:], scale_all[:, mt : mt + 1]
                        )
                    if apply_bb:
                        s = slice(h * PSJ, (h + 1) * PSJ)
                        nc.vector.tensor_tensor(
                            oc[:], oc[:], beta_bc[:, s], mybir.AluOpType.mult
                        )
                        nc.vector.tensor_tensor(
                            oc[:], oc[:], bbeta_bc[:, s], mybir.AluOpType.add
                        )
                    nc.sync.dma_start(
                        out_d[mt * P : (mt + 1) * P, h * PSJ : (h + 1) * PSJ], oc[:]
                    )

        if loop_n:
            with tc.For_i(0, loop_n, 1, hint_engines=(mybir.EngineType.PE,)):
                body()
        else:
            body()
    dedup_ldweights(nc)
    nc.compile()
    return nc


def _host_prep(input, weight, bias, gamma, beta):
    """Choose path and build per-core inputs.  Host work is layout/dtype-only:
    transposes plus sign-preserving casts; every reduction and sign() runs on
    device."""
    import ml_dtypes

    gamma = np.asarray(gamma, np.float32)
    bias = np.asarray(bias, np.float32)
    beta = np.asarray(beta, np.float32)
    input = np.asarray(input, np.float32)
    weight = np.asarray(weight, np.float32)

    fold_gamma = not np.all(gamma == 1.0)
    apply_bb = not (np.all(bias == 0.0) and np.all(beta == 1.0))

    if not fold_gamma and not apply_bb:
        # fast path
        N = input.shape[0]
        M = N // N_CORES
        x16 = input.astype(ml_dtypes.bfloat16)
        xTs = [
            np.ascontiguousarray(x16[c * M : (c + 1) * M].T) for c in range(N_CORES)
        ]
        wT = np.ascontiguousarray(weight.T.astype(ml_dtypes.bfloat16))
        in_maps = [{"xT": xTs[c], "wT": wT} for c in range(N_CORES)]
        return {"path": "fast", "in_maps": in_maps, "M": M,
                "K": input.shape[1], "J": weight.shape[0]}

    # general path
    input = np.ascontiguousarray(input)
    mode = "bf16" if fold_gamma else "fp8"
    wT = np.ascontiguousarray(weight.T.astype(ml_dtypes.bfloat16))
    extras = {}
    if fold_gamma:
        extras["ig"] = np.ascontiguousarray((1.0 / gamma)[:, None])
    if apply_bb:
        extras["bb"] = np.ascontiguousarray(
            np.stack([beta, bias * beta]).astype(np.float32)
        )
    N = input.shape[0]
    M = N // N_CORES
    in_maps = [
        {"x": np.ascontiguousarray(input[c * M : (c + 1) * M]), "wT": wT, **extras}
        for c in range(N_CORES)
    ]
    return {"path": "general", "in_maps": in_maps, "M": M, "K": input.shape[1],
            "J": weight.shape[0], "mode": mode, "fold_gamma": fold_gamma,
            "apply_bb": apply_bb}


def prep_and_build(input, weight, bias, gamma, beta, loop_n=0, unroll_n=0):
    """Returns (nc, in_maps). The program's 'out' outputs concatenate to the
    full [N, OUT_F] result (upcast to f32 by the caller)."""
    prep = _host_prep(input, weight, bias, gamma, beta)
    if prep["path"] == "fast":
        nc = build_fast(prep["M"], prep["K"], prep["J"], loop_n=loop_n,
                        unroll_n=unroll_n)
    else:
        nc = build_program(
            prep["M"], prep["K"], prep["J"], mode=prep["mode"],
            fold_gamma=prep["fold_gamma"], apply_bb=prep["apply_bb"],
            loop_n=loop_n,
        )
    return nc, prep["in_maps"]


def kernel(input, weight, bias, gamma, beta):
    nc, in_maps = prep_and_build(input, weight, bias, gamma, beta)

    from concourse.bass_utils import run_bass_kernel_spmd

    res = run_bass_kernel_spmd(nc, in_maps, list(range(N_CORES)))
    out = np.concatenate([r["out"] for r in res.results], axis=0)
    return np.ascontiguousarray(out.astype(np.float32))


if __name__ == "__main__":
    x = np.random.randn(1024, 512).astype(np.float32)
    w = np.random.randn(512, 512).astype(np.float32) * 0.01
    print(_host_prep(x, w, np.zeros(512), np.ones(512), np.ones(512))["path"])


# revision 13
# speedup vs baseline: 11.2282x; 1.0074x over previous
"""BitLinear Trainium2 kernel.

Computes, for input [N, IN_F], weight [OUT_F, IN_F], bias/beta [OUT_F], gamma [IN_F]:
    scale_i = max_k |input[i, k]|                         (per-row quant scale)
    out[i, j] = sum_k sign(input[i,k]) * (scale_i / gamma[k]) * sign(weight[j,k])
    out = (out + bias) * beta

Strategy: data-parallel shard input rows across 8 NeuronCores; every core holds
the full weight.  Host work is layout/dtype-only (transpose + sign-preserving
casts); all reductions/sign/matmul run on device.

Fast path (gamma==1, bias==0, beta==1 -- the shipped problem):
  - host sends xT = x.T as bf16 (sign bits and the per-row max-abs survive the
    bf16 cast to ~0.2%) and wT = w.T as bf16 (sign-exact cast; ACT sign with
    an fp8 INPUT produces NaN on real HW, so the weight stream stays bf16
    until the device-side sign writes the fp8 +-1 plane).
  - device: ACT signs both streams into fp8 +-1 planes; DVE computes the
    per-row abs-max with one strided tensor_reduce over the kt axis of the
    resident xT (layout [k, m]) followed by 8 tiny PE transposes + per-block
    max-reduces to land scale on the output row partitions; PE runs the sign matmul in fp8 DoubleRow (exact: +-1 operands,
    integer accumulation in fp32 PSUM); eviction applies scale as a
    per-partition scalar multiply and stores bf16 (host upcasts).
  - LDWEIGHTS dedup: the Tile scheduler emits one Ldweights per matmul even
    when consecutive matmuls share the stationary operand; a post-pass drops
    the redundant reloads (verified on HW: walrus pairs each MMUL with the
    nearest preceding LDW, and the PE array retains weights across MMULs).

General path (nonuniform gamma / bias / beta): previous-generation kernel,
bf16 matmul with 1/gamma folded into the quantized weight.
"""

import os
import sys
import numpy as np
from contextlib import ExitStack

sys.path.insert(0, "/opt/trn_rl_repo")

N_FULL, IN_F, OUT_F = 8192, 2048, 2048
N_CORES = 8
P = 128
NJ = 512  # matmul output column chunk (one PSUM bank)
PSJ = 1024  # psum tile width (2 banks)


def dedup_ldweights(nc):
    """Drop back-to-back InstLdweights with identical weight APs.

    Walrus lowers each InstMatmult's MMUL from the nearest preceding
    InstLdweights and the PE array keeps the stationary operand across
    matmuls, so an identical reload is pure overhead (~220 cycles each in
    DoubleRow).  fp32 matmuls self-load at the ISA level, so skip those.
    """
    from concourse import mybir

    SAFE = (mybir.InstMatmult, mybir.InstEventSemaphore, mybir.InstDrain)
    f32_dtypes = (mybir.dt.float32, mybir.dt.float32r)
    n_removed = 0
    for blk in nc.m.functions[0].blocks:
        insts = blk.instructions
        out = []
        changed = False
        last_key = None
        for inst in insts:
            if isinstance(inst, mybir.InstLdweights):
                ap = inst.ins[0]
                if getattr(ap, "dtype", None) in f32_dtypes:
                    last_key = None
                    out.append(inst)
                    continue
                key = (
                    repr(inst.ins),
                    str(inst.perf_mode),
                    str(inst.is_transpose),
                    str(inst.tile_position),
                    str(inst.tile_size),
                )
                if key == last_key:
                    n_removed += 1
                    changed = True
                    continue
                last_key = key
            elif not isinstance(inst, SAFE):
                last_key = None
            out.append(inst)
        if changed:
            blk.instructions = out
    return n_removed


def build_fast(M, K, J, loop_n=0, unroll_n=0):
    """Single-core Bass program for the gamma==1, no-bias/beta BitLinear shard.

    DRAM inputs: xT [K, M] bf16 (pre-transposed activations), wT [K, J] bf16
    (pre-transposed weight).  Output: out [M, J] bf16.
    """
    import concourse.bass as bass
    import concourse.tile as tile
    from concourse import bacc, mybir
    from concourse.masks import make_identity

    assert M % P == 0 and K % (2 * P) == 0 and J % PSJ == 0
    n_mt, n_kt = M // P, K // P
    n_g = n_kt // 2
    n_half = J // PSJ
    fp8 = mybir.dt.float8e4
    bf16 = mybir.dt.bfloat16
    f32 = mybir.dt.float32

    nc = bacc.Bacc("TRN2", target_bir_lowering=False, debug=False)
    xT_d = nc.dram_tensor("xT", [K, M], bf16, kind="ExternalInput")
    wT_d = nc.dram_tensor("wT", [K, J], bf16, kind="ExternalInput")
    out_d = nc.dram_tensor("out", [M, J], bf16, kind="ExternalOutput")

    with tile.TileContext(nc) as tc, ExitStack() as ctx:
        const_pool = ctx.enter_context(tc.tile_pool(name="const", bufs=1))
        xt_pool = ctx.enter_context(tc.tile_pool(name="xt", bufs=1))
        axt_pool = ctx.enter_context(tc.tile_pool(name="axt", bufs=2))
        wq_pool = ctx.enter_context(tc.tile_pool(name="wq", bufs=2))
        wraw_pool = ctx.enter_context(tc.tile_pool(name="wraw", bufs=3))
        macc_pool = ctx.enter_context(tc.tile_pool(name="macc", bufs=2))
        scale_pool = ctx.enter_context(tc.tile_pool(name="scalep", bufs=2))
        out_pool = ctx.enter_context(tc.tile_pool(name="outp", bufs=3))
        psum_pool = ctx.enter_context(tc.tile_pool(name="psum", bufs=3, space="PSUM"))
        tps_pool = ctx.enter_context(tc.tile_pool(name="tps", bufs=1, space="PSUM"))

        ident = const_pool.tile([P, P], bf16, name="ident")
        make_identity(nc, ident)

        def body():
            XT = xt_pool.tile([P, n_kt * M], bf16, name="XT", tag="XT")
            AXT = axt_pool.tile([P, n_kt * M], fp8, name="AXT", tag="AXT")
            WQ = wq_pool.tile([P, n_kt * J], fp8, name="WQ", tag="WQ")
            macc = macc_pool.tile([P, M], bf16, name="macc", tag="macc")
            scale_all = scale_pool.tile([P, n_mt], f32, name="scale", tag="scale")

            # ---- streams: one DMA + one sign per k-PAIR (the matmul
            # consumption granularity), for both operands ----
            for g in range(n_g):
                xs = XT[:, 2 * g * M : (2 * g + 2) * M]
                nc.sync.dma_start(
                    xs.rearrange("p (two m) -> p two m", two=2),
                    xT_d[2 * g * P : (2 * g + 2) * P, :].rearrange(
                        "(two p) m -> p two m", two=2
                    ),
                )
                wr = wraw_pool.tile([P, 2 * J], bf16, name=f"wr{g}", tag="wr")
                nc.sync.dma_start(
                    wr[:].rearrange("p (two j) -> p two j", two=2),
                    wT_d[2 * g * P : (2 * g + 2) * P, :].rearrange(
                        "(two p) j -> p two j", two=2
                    ),
                )
                nc.scalar.sign(AXT[:, 2 * g * M : (2 * g + 2) * M], xs)
                nc.scalar.sign(WQ[:, 2 * g * J : (2 * g + 2) * J], wr[:])

            # macc[p, m] = max_kt |XT[p, kt*M + m]| -- one strided reduce over
            # the kt axis (innermost after the view permutation).
            nc.vector.tensor_reduce(
                macc[:],
                XT[:].rearrange("p (kt m) -> p m kt", kt=n_kt),
                axis=mybir.AxisListType.X,
                op=mybir.AluOpType.max,
                apply_absolute_value=True,
            )

            # ---- per-row scale: macc [k-lane, m] -> scale_all [m-part, mt] ----
            tp = tps_pool.tile([P, M], bf16, name="tp", tag="tp")
            for mb in range(n_mt):
                nc.tensor.transpose(
                    tp[:, mb * P : (mb + 1) * P],
                    macc[:, mb * P : (mb + 1) * P],
                    ident[:],
                )
            for mb in range(n_mt):
                nc.vector.tensor_reduce(
                    scale_all[:, mb : mb + 1],
                    tp[:, mb * P : (mb + 1) * P],
                    axis=mybir.AxisListType.X,
                    op=mybir.AluOpType.max,
                )

            # ---- matmuls (m-outer, k-pair accumulation), scaled eviction ----
            for mt in range(n_mt):
                halves = [
                    psum_pool.tile([P, PSJ], f32, name=f"ps{mt}_{h}", tag="ps")
                    for h in range(n_half)
                ]
                for g in range(n_g):
                    lhsT = (
                        AXT[:, 2 * g * M : (2 * g + 2) * M]
                        .rearrange("p (two m) -> p two m", two=2)[
                            :, :, mt * P : (mt + 1) * P
                        ]
                    )
                    rhs3 = WQ[:, 2 * g * J : (2 * g + 2) * J].rearrange(
                        "p (two j) -> p two j", two=2
                    )
                    for nj in range(J // NJ):
                        ph, off = halves[nj // 2], (nj % 2) * NJ
                        nc.tensor.matmul(
                            ph[:, off : off + NJ],
                            lhsT=lhsT,
                            rhs=rhs3[:, :, nj * NJ : (nj + 1) * NJ],
                            start=(g == 0),
                            stop=(g == n_g - 1),
                            perf_mode=mybir.MatmulPerfMode.DoubleRow,
                        )
                # two m-tiles share one oc tile and one store DMA
                if mt % 2 == 0:
                    oc = out_pool.tile([P, 2 * J], bf16, name=f"oc{mt}", tag="oc")
                sub = oc[:, (mt % 2) * J : (mt % 2 + 1) * J]
                for h in range(n_half):
                    dst = sub[:, h * PSJ : (h + 1) * PSJ]
                    if h % 2:
                        nc.scalar.mul(dst, halves[h][:], scale_all[:, mt : mt + 1])
                    else:
                        nc.vector.tensor_scalar_mul(
                            dst, halves[h][:], scale_all[:, mt : mt + 1]
                        )
                if mt % 2 == 1:
                    # outputs ride the ACT HWDGE ring so input prefetch on the
                    # SP ring is never head-of-line blocked behind stores.
                    nc.scalar.dma_start(
                        out_d[(mt - 1) * P : (mt + 1) * P, :].rearrange(
                            "(two p) j -> p two j", two=2
                        ),
                        oc[:].rearrange("p (two j) -> p two j", two=2),
                    )

        if loop_n:
            # unroll_n bodies per hardware-loop iteration: consecutive bodies
            # pipeline through the rotating tile pools, so the per-iteration
            # scheduling barrier amortizes over unroll_n kernel executions.
            with tc.For_i(0, loop_n, 1, hint_engines=(mybir.EngineType.PE,)):
                for _ in range(max(1, unroll_n)):
                    body()
        elif unroll_n:
            for _ in range(unroll_n):
                body()
        else:
            body()
    dedup_ldweights(nc)
    nc.compile()
    return nc


def build_program(M, K, J, mode="fp8", fold_gamma=False, apply_bb=False, loop_n=0):
    """General-path program (previous generation): x [M, K] f32 row-major,
    wT [K, J] bf16, optional ig [K, 1] f32 and bb [2, J] f32."""
    import concourse.bass as bass
    import concourse.tile as tile
    from concourse import bacc, mybir
    from concourse.masks import make_identity

    assert M % P == 0 and K % P == 0 and J % PSJ == 0
    n_mt, n_kt, n_nj = M // P, K // P, J // NJ
    fp8 = mode == "fp8"
    if fp8:
        assert not fold_gamma and n_kt % 2 == 0
    cdt = mybir.dt.float8e4 if fp8 else mybir.dt.bfloat16
    f32 = mybir.dt.float32
    bf16 = mybir.dt.bfloat16

    nc = bacc.Bacc("TRN2", target_bir_lowering=False, debug=False)
    x_d = nc.dram_tensor("x", [M, K], f32, kind="ExternalInput")
    wT_d = nc.dram_tensor("wT", [K, J], bf16, kind="ExternalInput")
    ig_d = (
        nc.dram_tensor("ig", [K, 1], f32, kind="ExternalInput") if fold_gamma else None
    )
    bb_d = (
        nc.dram_tensor("bb", [2, J], f32, kind="ExternalInput") if apply_bb else None
    )
    odt = f32 if apply_bb else bf16
    out_d = nc.dram_tensor("out", [M, J], odt, kind="ExternalOutput")

    with tile.TileContext(nc) as tc, ExitStack() as ctx:
        aT_pool = ctx.enter_context(tc.tile_pool(name="aT", bufs=1))
        act_pool = ctx.enter_context(tc.tile_pool(name="act", bufs=3))
        asg_pool = ctx.enter_context(tc.tile_pool(name="asg", bufs=2))
        const_pool = ctx.enter_context(tc.tile_pool(name="const", bufs=1))
        scale_pool = ctx.enter_context(tc.tile_pool(name="scalep", bufs=1))
        wraw_pool = ctx.enter_context(tc.tile_pool(name="wraw", bufs=3))
        wq_pool = ctx.enter_context(tc.tile_pool(name="wq", bufs=1))
        out_pool = ctx.enter_context(tc.tile_pool(name="outp", bufs=3))
        psum_pool = ctx.enter_context(tc.tile_pool(name="psum", bufs=3, space="PSUM"))
        tpsum_pool = ctx.enter_context(tc.tile_pool(name="tpsum", bufs=1, space="PSUM"))

        ident = const_pool.tile([P, P], bf16, name="ident")
        make_identity(nc, ident)

        beta_bc = bbeta_bc = None
        if apply_bb:
            beta_bc = const_pool.tile([P, J], f32, name="beta_bc")
            bbeta_bc = const_pool.tile([P, J], f32, name="bbeta_bc")
            nc.sync.dma_start(beta_bc[:], bb_d[0:1, :].broadcast_to([P, J]))
            nc.sync.dma_start(bbeta_bc[:], bb_d[1:2, :].broadcast_to([P, J]))

        igs = []
        if fold_gamma:
            for kt in range(n_kt):
                ig_t = const_pool.tile([P, 1], f32, name=f"ig{kt}", tag=f"ig{kt}")
                nc.sync.dma_start(ig_t[:], ig_d[kt * P : (kt + 1) * P, :])
                igs.append(ig_t)

        def body():
            scale_all = scale_pool.tile([P, n_mt], f32, name="scale_all")
            aTs = []
            wqs = []

            def emit_weight(kt):
                wr = wraw_pool.tile([P, J], bf16, name=f"wr{kt}", tag="wr")
                nc.sync.dma_start(wr[:], wT_d[kt * P : (kt + 1) * P, :])
                if fp8:
                    if kt % 2 == 0:
                        wqs.append(
                            wq_pool.tile(
                                [P, 2 * J], cdt, name=f"wq{kt // 2}", tag=f"wq{kt // 2}"
                            )
                        )
                    dst = wqs[-1][:, (kt % 2) * J : (kt % 2 + 1) * J]
                    nc.scalar.sign(dst, wr[:])
                else:
                    wq_t = wq_pool.tile([P, J], cdt, name=f"wq{kt}", tag=f"wq{kt}")
                    nc.scalar.sign(wq_t[:], wr[:])
                    if fold_gamma:
                        nc.vector.tensor_scalar_mul(wq_t[:], wq_t[:], igs[kt][:, 0:1])
                    wqs.append(wq_t)

            def emit_act(mt):
                a_raw = act_pool.tile([P, K], f32, name=f"a_raw{mt}", tag="a_raw")
                nc.sync.dma_start(a_raw[:], x_d[mt * P : (mt + 1) * P, :])
                nc.vector.tensor_reduce(
                    scale_all[:, mt : mt + 1],
                    a_raw[:],
                    axis=mybir.AxisListType.X,
                    op=mybir.AluOpType.max,
                    apply_absolute_value=True,
                )
                asg = asg_pool.tile([P, K], bf16, name=f"asg{mt}", tag="asg")
                nc.scalar.sign(asg[:], a_raw[:])
                tp = tpsum_pool.tile([P, K], bf16, name=f"tp{mt}", tag="tp")
                for kt in range(n_kt):
                    nc.tensor.transpose(
                        tp[:, kt * P : (kt + 1) * P],
                        asg[:, kt * P : (kt + 1) * P],
                        ident[:],
                    )
                aT = aT_pool.tile([P, K], cdt, name=f"aT{mt}", tag=f"aT{mt}")
                nc.vector.tensor_copy(aT[:], tp[:])
                aTs.append(aT)

            k_per_mt = (n_kt + n_mt - 1) // n_mt
            ki = 0
            for mt in range(n_mt):
                emit_act(mt)
                for _ in range(k_per_mt):
                    if ki < n_kt:
                        emit_weight(ki)
                        ki += 1
            while ki < n_kt:
                emit_weight(ki)
                ki += 1

            n_half = J // PSJ
            n_groups = n_kt // 2 if fp8 else n_kt

            def emit_mms(mt, g0, g1, start):
                halves = [
                    psum_pool.tile([P, PSJ], f32, name=f"ps{mt}_{g0}_{h}", tag="ps")
                    for h in range(n_half)
                ]
                for g in range(g0, g1):
                    if fp8:
                        lhsT = aTs[mt][:, g * 256 : (g + 1) * 256].rearrange(
                            "p (two m) -> p two m", two=2
                        )
                        rhs3 = wqs[g][:].rearrange("p (two j) -> p two j", two=2)
                    else:
                        lhsT = aTs[mt][:, g * P : (g + 1) * P]
                    for nj in range(n_nj):
                        ph, off = halves[nj // 2], (nj % 2) * NJ
                        if fp8:
                            nc.tensor.matmul(
                                ph[:, off : off + NJ],
                                lhsT=lhsT,
                                rhs=rhs3[:, :, nj * NJ : (nj + 1) * NJ],
                                start=(g == g0),
                                stop=(g == g1 - 1),
                                perf_mode=mybir.MatmulPerfMode.DoubleRow,
                            )
                        else:
                            nc.tensor.matmul(
                                ph[:, off : off + NJ],
                                lhsT=lhsT,
                                rhs=wqs[g][:, nj * NJ : (nj + 1) * NJ],
                                start=(g == g0),
                                stop=(g == g1 - 1),
                            )
                return halves

            for mt in range(n_mt):
                halves = emit_mms(mt, 0, n_groups, True)
                for h in range(n_half):
                    oc = out_pool.tile([P, PSJ], odt, name=f"oc{mt}_{h}", tag="oc")
                    if h % 2:
                        nc.scalar.mul(oc[:], halves[h][:], scale_all[:, mt : mt + 1])
                    else:
                        nc.vector.tensor_scalar_mul(
                            oc[:], halves[h][:], scale_all[:, mt : mt + 1]
                        )
                    if apply_bb:
                        s = slice(h * PSJ, (h + 1) * PSJ)
                        nc.vector.tensor_tensor(
                            oc[:], oc[:], beta_bc[:, s], mybir.AluOpType.mult
                        )
                        nc.vector.tensor_tensor(
                            oc[:], oc[:], bbeta_bc[:, s], mybir.AluOpType.add
                        )
                    nc.sync.dma_start(
                        out_d[mt * P : (mt + 1) * P, h * PSJ : (h + 1) * PSJ], oc[:]
                    )

        if loop_n:
            with tc.For_i(0, loop_n, 1, hint_engines=(mybir.EngineType.PE,)):
                body()
        else:
            body()
    dedup_ldweights(nc)
    nc.compile()
    return nc


def _host_prep(input, weight, bias, gamma, beta):
    """Choose path and build per-core inputs.  Host work is layout/dtype-only:
    transposes plus sign-preserving casts; every reduction and sign() runs on
    device."""
    import ml_dtypes

    gamma = np.asarray(gamma, np.float32)
    bias = np.asarray(bias, np.float32)
    beta = np.asarray(beta, np.float32)
    input = np.asarray(input, np.float32)
    weight = np.asarray(weight, np.float32)

    fold_gamma = not np.all(gamma == 1.0)
    apply_bb = not (np.all(bias == 0.0) and np.all(beta == 1.0))

    if not fold_gamma and not apply_bb:
        # fast path
        N = input.shape[0]
        M = N // N_CORES
        x16 = input.astype(ml_dtypes.bfloat16)
        xTs = [
            np.ascontiguousarray(x16[c * M : (c + 1) * M].T) for c in range(N_CORES)
        ]
        wT = np.ascontiguousarray(weight.T.astype(ml_dtypes.bfloat16))
        in_maps = [{"xT": xTs[c], "wT": wT} for c in range(N_CORES)]
        return {"path": "fast", "in_maps": in_maps, "M": M,
                "K": input.shape[1], "J": weight.shape[0]}

    # general path
    input = np.ascontiguousarray(input)
    mode = "bf16" if fold_gamma else "fp8"
    wT = np.ascontiguousarray(weight.T.astype(ml_dtypes.bfloat16))
    extras = {}
    if fold_gamma:
        extras["ig"] = np.ascontiguousarray((1.0 / gamma)[:, None])
    if apply_bb:
        extras["bb"] = np.ascontiguousarray(
            np.stack([beta, bias * beta]).astype(np.float32)
        )
    N = input.shape[0]
    M = N // N_CORES
    in_maps = [
        {"x": np.ascontiguousarray(input[c * M : (c + 1) * M]), "wT": wT, **extras}
        for c in range(N_CORES)
    ]
    return {"path": "general", "in_maps": in_maps, "M": M, "K": input.shape[1],
            "J": weight.shape[0], "mode": mode, "fold_gamma": fold_gamma,
            "apply_bb": apply_bb}


def prep_and_build(input, weight, bias, gamma, beta, loop_n=0, unroll_n=0):
    """Returns (nc, in_maps). The program's 'out' outputs concatenate to the
    full [N, OUT_F] result (upcast to f32 by the caller)."""
    prep = _host_prep(input, weight, bias, gamma, beta)
    if prep["path"] == "fast":
        nc = build_fast(prep["M"], prep["K"], prep["J"], loop_n=loop_n,
                        unroll_n=unroll_n)
    else:
        nc = build_program(
            prep["M"], prep["K"], prep["J"], mode=prep["mode"],
            fold_gamma=prep["fold_gamma"], apply_bb=prep["apply_bb"],
            loop_n=loop_n,
        )
    return nc, prep["in_maps"]


def kernel(input, weight, bias, gamma, beta):
    nc, in_maps = prep_and_build(input, weight, bias, gamma, beta)

    from concourse.bass_utils import run_bass_kernel_spmd

    res = run_bass_kernel_spmd(nc, in_maps, list(range(N_CORES)))
    out = np.concatenate([r["out"] for r in res.results], axis=0)
    return np.ascontiguousarray(out.astype(np.float32))


if __name__ == "__main__":
    x = np.random.randn(1024, 512).astype(np.float32)
    w = np.random.randn(512, 512).astype(np.float32) * 0.01
    print(_host_prep(x, w, np.zeros(512), np.ones(512), np.ones(512))["path"])


# revision 14
# speedup vs baseline: 12.1326x; 1.0806x over previous
"""BitLinear Trainium2 kernel.

Computes, for input [N, IN_F], weight [OUT_F, IN_F], bias/beta [OUT_F], gamma [IN_F]:
    scale_i = max_k |input[i, k]|                         (per-row quant scale)
    out[i, j] = sum_k sign(input[i,k]) * (scale_i / gamma[k]) * sign(weight[j,k])
    out = (out + bias) * beta

Strategy: data-parallel shard input rows across 8 NeuronCores; every core holds
the full weight.  Host work is layout/dtype-only (transpose + sign-preserving
casts); all reductions/sign/matmul run on device.

Fast path (gamma==1, bias==0, beta==1 -- the shipped problem):
  - host sends xT = x.T as bf16 (sign bits and the per-row max-abs survive the
    bf16 cast to ~0.2%) and wT = w.T as bf16 (sign-exact cast; ACT sign with
    an fp8 INPUT produces NaN on real HW, so the weight stream stays bf16
    until the device-side sign writes the fp8 +-1 plane).
  - device: ACT signs both streams into fp8 +-1 planes; DVE computes the
    per-row abs-max with one strided tensor_reduce over the kt axis of the
    resident xT (layout [k, m]) followed by 8 tiny PE transposes + per-block
    max-reduces to land scale on the output row partitions; PE runs the sign matmul in fp8 DoubleRow (exact: +-1 operands,
    integer accumulation in fp32 PSUM); eviction applies scale as a
    per-partition scalar multiply and stores bf16 (host upcasts).
  - LDWEIGHTS dedup: the Tile scheduler emits one Ldweights per matmul even
    when consecutive matmuls share the stationary operand; a post-pass drops
    the redundant reloads (verified on HW: walrus pairs each MMUL with the
    nearest preceding LDW, and the PE array retains weights across MMULs).

General path (nonuniform gamma / bias / beta): previous-generation kernel,
bf16 matmul with 1/gamma folded into the quantized weight.
"""

import os
import sys
import numpy as np
from contextlib import ExitStack

sys.path.insert(0, "/opt/trn_rl_repo")

N_FULL, IN_F, OUT_F = 8192, 2048, 2048
N_CORES = 8
P = 128
NJ = 512  # matmul output column chunk (one PSUM bank)
PSJ = 1024  # psum tile width (2 banks)


def dedup_ldweights(nc):
    """Drop back-to-back InstLdweights with identical weight APs.

    Walrus lowers each InstMatmult's MMUL from the nearest preceding
    InstLdweights and the PE array keeps the stationary operand across
    matmuls, so an identical reload is pure overhead (~220 cycles each in
    DoubleRow).  fp32 matmuls self-load at the ISA level, so skip those.
    """
    from concourse import mybir

    SAFE = (mybir.InstMatmult, mybir.InstEventSemaphore, mybir.InstDrain)
    f32_dtypes = (mybir.dt.float32, mybir.dt.float32r)
    n_removed = 0
    for blk in nc.m.functions[0].blocks:
        insts = blk.instructions
        out = []
        changed = False
        last_key = None
        for inst in insts:
            if isinstance(inst, mybir.InstLdweights):
                ap = inst.ins[0]
                if getattr(ap, "dtype", None) in f32_dtypes:
                    last_key = None
                    out.append(inst)
                    continue
                key = (
                    repr(inst.ins),
                    str(inst.perf_mode),
                    str(inst.is_transpose),
                    str(inst.tile_position),
                    str(inst.tile_size),
                )
                if key == last_key:
                    n_removed += 1
                    changed = True
                    continue
                last_key = key
            elif not isinstance(inst, SAFE):
                last_key = None
            out.append(inst)
        if changed:
            blk.instructions = out
    return n_removed


def build_fast(M, K, J, loop_n=0, unroll_n=0):
    """Single-core Bass program for the gamma==1, no-bias/beta BitLinear shard.

    DRAM inputs: xT [K, M] bf16 (pre-transposed activations), wT [K, J] bf16
    (pre-transposed weight).  Output: out [M, J] bf16.
    """
    import concourse.bass as bass
    import concourse.tile as tile
    from concourse import bacc, mybir
    from concourse.masks import make_identity

    assert M % P == 0 and K % (2 * P) == 0 and J % PSJ == 0
    n_mt, n_kt = M // P, K // P
    n_g = n_kt // 2
    n_half = J // PSJ
    fp8 = mybir.dt.float8e4
    bf16 = mybir.dt.bfloat16
    f32 = mybir.dt.float32

    nc = bacc.Bacc("TRN2", target_bir_lowering=False, debug=False)
    xT_d = nc.dram_tensor("xT", [K, M], bf16, kind="ExternalInput")
    wT_d = nc.dram_tensor("wT", [K, J], bf16, kind="ExternalInput")
    out_d = nc.dram_tensor("out", [M, J], bf16, kind="ExternalOutput")

    with tile.TileContext(nc) as tc, ExitStack() as ctx:
        const_pool = ctx.enter_context(tc.tile_pool(name="const", bufs=1))
        xt_pool = ctx.enter_context(tc.tile_pool(name="xt", bufs=1))
        axt_pool = ctx.enter_context(tc.tile_pool(name="axt", bufs=2))
        wq_pool = ctx.enter_context(tc.tile_pool(name="wq", bufs=2))
        wraw_pool = ctx.enter_context(tc.tile_pool(name="wraw", bufs=3))
        macc_pool = ctx.enter_context(tc.tile_pool(name="macc", bufs=2))
        scale_pool = ctx.enter_context(tc.tile_pool(name="scalep", bufs=2))
        out_pool = ctx.enter_context(tc.tile_pool(name="outp", bufs=3))
        psum_pool = ctx.enter_context(tc.tile_pool(name="psum", bufs=3, space="PSUM"))
        tps_pool = ctx.enter_context(tc.tile_pool(name="tps", bufs=1, space="PSUM"))

        ident = const_pool.tile([P, P], bf16, name="ident")
        make_identity(nc, ident)

        def body():
            XT = xt_pool.tile([P, n_kt * M], bf16, name="XT", tag="XT")
            AXT = axt_pool.tile([P, n_kt * M], fp8, name="AXT", tag="AXT")
            WQ = wq_pool.tile([P, n_kt * J], fp8, name="WQ", tag="WQ")
            macc = macc_pool.tile([P, M], bf16, name="macc", tag="macc")
            scale_all = scale_pool.tile([P, n_mt], f32, name="scale", tag="scale")

            # ---- streams: one DMA + one sign per k-PAIR (the matmul
            # consumption granularity), for both operands ----
            for g in range(n_g):
                xs = XT[:, 2 * g * M : (2 * g + 2) * M]
                nc.sync.dma_start(
                    xs.rearrange("p (two m) -> p two m", two=2),
                    xT_d[2 * g * P : (2 * g + 2) * P, :].rearrange(
                        "(two p) m -> p two m", two=2
                    ),
                )
                wr = wraw_pool.tile([P, 2 * J], bf16, name=f"wr{g}", tag="wr")
                nc.sync.dma_start(
                    wr[:].rearrange("p (two j) -> p two j", two=2),
                    wT_d[2 * g * P : (2 * g + 2) * P, :].rearrange(
                        "(two p) j -> p two j", two=2
                    ),
                )
                nc.scalar.sign(AXT[:, 2 * g * M : (2 * g + 2) * M], xs)
                nc.scalar.sign(WQ[:, 2 * g * J : (2 * g + 2) * J], wr[:])

            # macc[p, m] = max_kt |XT[p, kt*M + m]| -- one strided reduce over
            # the kt axis (innermost after the view permutation).
            nc.vector.tensor_reduce(
                macc[:],
                XT[:].rearrange("p (kt m) -> p m kt", kt=n_kt),
                axis=mybir.AxisListType.X,
                op=mybir.AluOpType.max,
                apply_absolute_value=True,
            )

            # ---- per-row scale: macc [k-lane, m] -> scale_all [m-part, mt] ----
            tp = tps_pool.tile([P, M], bf16, name="tp", tag="tp")
            for mb in range(n_mt):
                nc.tensor.transpose(
                    tp[:, mb * P : (mb + 1) * P],
                    macc[:, mb * P : (mb + 1) * P],
                    ident[:],
                )
            for mb in range(n_mt):
                nc.vector.tensor_reduce(
                    scale_all[:, mb : mb + 1],
                    tp[:, mb * P : (mb + 1) * P],
                    axis=mybir.AxisListType.X,
                    op=mybir.AluOpType.max,
                )

            # ---- matmuls (m-outer, k-pair accumulation), scaled eviction ----
            for mt in range(n_mt):
                halves = [
                    psum_pool.tile([P, PSJ], f32, name=f"ps{mt}_{h}", tag="ps")
                    for h in range(n_half)
                ]
                for g in range(n_g):
                    lhsT = (
                        AXT[:, 2 * g * M : (2 * g + 2) * M]
                        .rearrange("p (two m) -> p two m", two=2)[
                            :, :, mt * P : (mt + 1) * P
                        ]
                    )
                    rhs3 = WQ[:, 2 * g * J : (2 * g + 2) * J].rearrange(
                        "p (two j) -> p two j", two=2
                    )
                    for nj in range(J // NJ):
                        ph, off = halves[nj // 2], (nj % 2) * NJ
                        nc.tensor.matmul(
                            ph[:, off : off + NJ],
                            lhsT=lhsT,
                            rhs=rhs3[:, :, nj * NJ : (nj + 1) * NJ],
                            start=(g == 0),
                            stop=(g == n_g - 1),
                            perf_mode=mybir.MatmulPerfMode.DoubleRow,
                        )
                # two m-tiles share one oc tile and one store DMA
                if mt % 2 == 0:
                    oc = out_pool.tile([P, 2 * J], bf16, name=f"oc{mt}", tag="oc")
                sub = oc[:, (mt % 2) * J : (mt % 2 + 1) * J]
                for h in range(n_half):
                    dst = sub[:, h * PSJ : (h + 1) * PSJ]
                    if h % 2:
                        nc.scalar.mul(dst, halves[h][:], scale_all[:, mt : mt + 1])
                    else:
                        nc.vector.tensor_scalar_mul(
                            dst, halves[h][:], scale_all[:, mt : mt + 1]
                        )
                if mt % 2 == 1:
                    # outputs ride the ACT HWDGE ring so input prefetch on the
                    # SP ring is never head-of-line blocked behind stores.
                    nc.scalar.dma_start(
                        out_d[(mt - 1) * P : (mt + 1) * P, :].rearrange(
                            "(two p) j -> p two j", two=2
                        ),
                        oc[:].rearrange("p (two j) -> p two j", two=2),
                    )

        if loop_n:
            # unroll_n bodies per hardware-loop iteration: consecutive bodies
            # pipeline through the rotating tile pools, so the per-iteration
            # scheduling barrier amortizes over unroll_n kernel executions.
            # staggered_reset lets engines reset loop state independently
            # instead of an all-engine barrier at each back-edge.
            stag = os.environ.get("STAGGERED", "1") == "1"
            with tc.For_i(
                0, loop_n, 1, hint_engines=(mybir.EngineType.PE,),
                staggered_reset=stag,
            ):
                for _ in range(max(1, unroll_n)):
                    body()
        elif unroll_n:
            for _ in range(unroll_n):
                body()
        else:
            body()
    dedup_ldweights(nc)
    nc.compile()
    return nc


def build_program(M, K, J, mode="fp8", fold_gamma=False, apply_bb=False, loop_n=0):
    """General-path program (previous generation): x [M, K] f32 row-major,
    wT [K, J] bf16, optional ig [K, 1] f32 and bb [2, J] f32."""
    import concourse.bass as bass
    import concourse.tile as tile
    from concourse import bacc, mybir
    from concourse.masks import make_identity

    assert M % P == 0 and K % P == 0 and J % PSJ == 0
    n_mt, n_kt, n_nj = M // P, K // P, J // NJ
    fp8 = mode == "fp8"
    if fp8:
        assert not fold_gamma and n_kt % 2 == 0
    cdt = mybir.dt.float8e4 if fp8 else mybir.dt.bfloat16
    f32 = mybir.dt.float32
    bf16 = mybir.dt.bfloat16

    nc = bacc.Bacc("TRN2", target_bir_lowering=False, debug=False)
    x_d = nc.dram_tensor("x", [M, K], f32, kind="ExternalInput")
    wT_d = nc.dram_tensor("wT", [K, J], bf16, kind="ExternalInput")
    ig_d = (
        nc.dram_tensor("ig", [K, 1], f32, kind="ExternalInput") if fold_gamma else None
    )
    bb_d = (
        nc.dram_tensor("bb", [2, J], f32, kind="ExternalInput") if apply_bb else None
    )
    odt = f32 if apply_bb else bf16
    out_d = nc.dram_tensor("out", [M, J], odt, kind="ExternalOutput")

    with tile.TileContext(nc) as tc, ExitStack() as ctx:
        aT_pool = ctx.enter_context(tc.tile_pool(name="aT", bufs=1))
        act_pool = ctx.enter_context(tc.tile_pool(name="act", bufs=3))
        asg_pool = ctx.enter_context(tc.tile_pool(name="asg", bufs=2))
        const_pool = ctx.enter_context(tc.tile_pool(name="const", bufs=1))
        scale_pool = ctx.enter_context(tc.tile_pool(name="scalep", bufs=1))
        wraw_pool = ctx.enter_context(tc.tile_pool(name="wraw", bufs=3))
        wq_pool = ctx.enter_context(tc.tile_pool(name="wq", bufs=1))
        out_pool = ctx.enter_context(tc.tile_pool(name="outp", bufs=3))
        psum_pool = ctx.enter_context(tc.tile_pool(name="psum", bufs=3, space="PSUM"))
        tpsum_pool = ctx.enter_context(tc.tile_pool(name="tpsum", bufs=1, space="PSUM"))

        ident = const_pool.tile([P, P], bf16, name="ident")
        make_identity(nc, ident)

        beta_bc = bbeta_bc = None
        if apply_bb:
            beta_bc = const_pool.tile([P, J], f32, name="beta_bc")
            bbeta_bc = const_pool.tile([P, J], f32, name="bbeta_bc")
            nc.sync.dma_start(beta_bc[:], bb_d[0:1, :].broadcast_to([P, J]))
            nc.sync.dma_start(bbeta_bc[:], bb_d[1:2, :].broadcast_to([P, J]))

        igs = []
        if fold_gamma:
            for kt in range(n_kt):
                ig_t = const_pool.tile([P, 1], f32, name=f"ig{kt}", tag=f"ig{kt}")
                nc.sync.dma_start(ig_t[:], ig_d[kt * P : (kt + 1) * P, :])
                igs.append(ig_t)

        def body():
            scale_all = scale_pool.tile([P, n_mt], f32, name="scale_all")
            aTs = []
            wqs = []

            def emit_weight(kt):
                wr = wraw_pool.tile([P, J], bf16, name=f"wr{kt}", tag="wr")
                nc.sync.dma_start(wr[:], wT_d[kt * P : (kt + 1) * P, :])
                if fp8:
                    if kt % 2 == 0:
                        wqs.append(
                            wq_pool.tile(
                                [P, 2 * J], cdt, name=f"wq{kt // 2}", tag=f"wq{kt // 2}"
                            )
                        )
                    dst = wqs[-1][:, (kt % 2) * J : (kt % 2 + 1) * J]
                    nc.scalar.sign(dst, wr[:])
                else:
                    wq_t = wq_pool.tile([P, J], cdt, name=f"wq{kt}", tag=f"wq{kt}")
                    nc.scalar.sign(wq_t[:], wr[:])
                    if fold_gamma:
                        nc.vector.tensor_scalar_mul(wq_t[:], wq_t[:], igs[kt][:, 0:1])
                    wqs.append(wq_t)

            def emit_act(mt):
                a_raw = act_pool.tile([P, K], f32, name=f"a_raw{mt}", tag="a_raw")
                nc.sync.dma_start(a_raw[:], x_d[mt * P : (mt + 1) * P, :])
                nc.vector.tensor_reduce(
                    scale_all[:, mt : mt + 1],
                    a_raw[:],
                    axis=mybir.AxisListType.X,
                    op=mybir.AluOpType.max,
                    apply_absolute_value=True,
                )
                asg = asg_pool.tile([P, K], bf16, name=f"asg{mt}", tag="asg")
                nc.scalar.sign(asg[:], a_raw[:])
                tp = tpsum_pool.tile([P, K], bf16, name=f"tp{mt}", tag="tp")
                for kt in range(n_kt):
                    nc.tensor.transpose(
                        tp[:, kt * P : (kt + 1) * P],
                        asg[:, kt * P : (kt + 1) * P],
                        ident[:],
                    )
                aT = aT_pool.tile([P, K], cdt, name=f"aT{mt}", tag=f"aT{mt}")
                nc.vector.tensor_copy(aT[:], tp[:])
                aTs.append(aT)

            k_per_mt = (n_kt + n_mt - 1) // n_mt
            ki = 0
            for mt in range(n_mt):
                emit_act(mt)
                for _ in range(k_per_mt):
                    if ki < n_kt:
                        emit_weight(ki)
                        ki += 1
            while ki < n_kt:
                emit_weight(ki)
                ki += 1

            n_half = J // PSJ
            n_groups = n_kt // 2 if fp8 else n_kt

            def emit_mms(mt, g0, g1, start):
                halves = [
                    psum_pool.tile([P, PSJ], f32, name=f"ps{mt}_{g0}_{h}", tag="ps")
                    for h in range(n_half)
                ]
                for g in range(g0, g1):
                    if fp8:
                        lhsT = aTs[mt][:, g * 256 : (g + 1) * 256].rearrange(
                            "p (two m) -> p two m", two=2
                        )
                        rhs3 = wqs[g][:].rearrange("p (two j) -> p two j", two=2)
                    else:
                        lhsT = aTs[mt][:, g * P : (g + 1) * P]
                    for nj in range(n_nj):
                        ph, off = halves[nj // 2], (nj % 2) * NJ
                        if fp8:
                            nc.tensor.matmul(
                                ph[:, off : off + NJ],
                                lhsT=lhsT,
                                rhs=rhs3[:, :, nj * NJ : (nj + 1) * NJ],
                                start=(g == g0),
                                stop=(g == g1 - 1),
                                perf_mode=mybir.MatmulPerfMode.DoubleRow,
                            )
                        else:
                            nc.tensor.matmul(
                                ph[:, off : off + NJ],
                                lhsT=lhsT,
                                rhs=wqs[g][:, nj * NJ : (nj + 1) * NJ],
                                start=(g == g0),
                                stop=(g == g1 - 1),
                            )
                return halves

            for mt in range(n_mt):
                halves = emit_mms(mt, 0, n_groups, True)
                for h in range(n_half):
                    oc = out_pool.tile([P, PSJ], odt, name=f"oc{mt}_{h}", tag="oc")
                    if h % 2:
                        nc.scalar.mul(oc[:], halves[h][:], scale_all[:, mt : mt + 1])
                    else:
                        nc.vector.tensor_scalar_mul(
                            oc[:], halves[h][:], scale_all[:, mt : mt + 1]
                        )
                    if apply_bb:
                        s = slice(h * PSJ, (h + 1) * PSJ)
                        nc.vector.tensor_tensor(
                            oc[:], oc[:], beta_bc[:, s], mybir.AluOpType.mult
                        )
                        nc.vector.tensor_tensor(
                            oc[:], oc[:], bbeta_bc[:, s], mybir.AluOpType.add
                        )
                    nc.sync.dma_start(
                        out_d[mt * P : (mt + 1) * P, h * PSJ : (h + 1) * PSJ], oc[:]
                    )

        if loop_n:
            with tc.For_i(0, loop_n, 1, hint_engines=(mybir.EngineType.PE,)):
                body()
        else:
            body()
    dedup_ldweights(nc)
    nc.compile()
    return nc


def _host_prep(input, weight, bias, gamma, beta):
    """Choose path and build per-core inputs.  Host work is layout/dtype-only:
    transposes plus sign-preserving casts; every reduction and sign() runs on
    device."""
    import ml_dtypes

    gamma = np.asarray(gamma, np.float32)
    bias = np.asarray(bias, np.float32)
    beta = np.asarray(beta, np.float32)
    input = np.asarray(input, np.float32)
    weight = np.asarray(weight, np.float32)

    fold_gamma = not np.all(gamma == 1.0)
    apply_bb = not (np.all(bias == 0.0) and np.all(beta == 1.0))

    if not fold_gamma and not apply_bb:
        # fast path
        N = input.shape[0]
        M = N // N_CORES
        x16 = input.astype(ml_dtypes.bfloat16)
        xTs = [
            np.ascontiguousarray(x16[c * M : (c + 1) * M].T) for c in range(N_CORES)
        ]
        wT = np.ascontiguousarray(weight.T.astype(ml_dtypes.bfloat16))
        in_maps = [{"xT": xTs[c], "wT": wT} for c in range(N_CORES)]
        return {"path": "fast", "in_maps": in_maps, "M": M,
                "K": input.shape[1], "J": weight.shape[0]}

    # general path
    input = np.ascontiguousarray(input)
    mode = "bf16" if fold_gamma else "fp8"
    wT = np.ascontiguousarray(weight.T.astype(ml_dtypes.bfloat16))
    extras = {}
    if fold_gamma:
        extras["ig"] = np.ascontiguousarray((1.0 / gamma)[:, None])
    if apply_bb:
        extras["bb"] = np.ascontiguousarray(
            np.stack([beta, bias * beta]).astype(np.float32)
        )
    N = input.shape[0]
    M = N // N_CORES
    in_maps = [
        {"x": np.ascontiguousarray(input[c * M : (c + 1) * M]), "wT": wT, **extras}
        for c in range(N_CORES)
    ]
    return {"path": "general", "in_maps": in_maps, "M": M, "K": input.shape[1],
            "J": weight.shape[0], "mode": mode, "fold_gamma": fold_gamma,
            "apply_bb": apply_bb}


def prep_and_build(input, weight, bias, gamma, beta, loop_n=0, unroll_n=0):
    """Returns (nc, in_maps). The program's 'out' outputs concatenate to the
    full [N, OUT_F] result (upcast to f32 by the caller)."""
    prep = _host_prep(input, weight, bias, gamma, beta)
    if prep["path"] == "fast":
        nc = build_fast(prep["M"], prep["K"], prep["J"], loop_n=loop_n,
                        unroll_n=unroll_n)
    else:
        nc = build_program(
            prep["M"], prep["K"], prep["J"], mode=prep["mode"],
            fold_gamma=prep["fold_gamma"], apply_bb=prep["apply_bb"],
            loop_n=loop_n,
        )
    return nc, prep["in_maps"]


def kernel(input, weight, bias, gamma, beta):
    nc, in_maps = prep_and_build(input, weight, bias, gamma, beta)

    from concourse.bass_utils import run_bass_kernel_spmd

    res = run_bass_kernel_spmd(nc, in_maps, list(range(N_CORES)))
    out = np.concatenate([r["out"] for r in res.results], axis=0)
    return np.ascontiguousarray(out.astype(np.float32))


if __name__ == "__main__":
    x = np.random.randn(1024, 512).astype(np.float32)
    w = np.random.randn(512, 512).astype(np.float32) * 0.01
    print(_host_prep(x, w, np.zeros(512), np.ones(512), np.ones(512))["path"])
